# revision 96
# baseline (speedup 1.0000x reference)
"""Trainium2 Bass kernel for a pre-norm transformer encoder layer.

Problem: x[2,2048,1024]; LN1 -> QKV (16 heads x 64) -> softmax(QK^T) V
-> wo -> +res -> LN2 -> GELU(h@w1+b1)@w2+b2 -> +res.

Sharding: token-parallel over B*N = 4096 tokens; each of the 8 cores owns
512 tokens (cores 0-3: batch 0, cores 4-7: batch 1). Each core recomputes
K/V for its whole batch (no collectives). All activations are kept in
transposed layout [feature, token] so every matmul contracts over the
partition dim. Host pre-rotates each core's batch so its own 512 tokens
are always columns 0:512 -> one NEFF shared by all 8 cores.

v3 restructure (fp8-DoubleRow on the error-tolerant matmuls, multi-engine
softmax, latency-pipelined phases):
 - LN gammas are folded into wq/wk/wv/w1 rows on the host; LN betas become
   per-output-feature constants (b@W) applied for free as per-partition
   bias APs in the PSUM->SBUF copies. Device LN is (x-mu)*rstd = x*A - B.
 - LN statistics (sum, sum-of-squares) both via all-ones [128,128]
   stationary PE matmul chains (replicated output); the chunk loop is
   software-pipelined so chunk c+1's stats sit in the (in-order) PE queue
   before chunk c's Q/K/V work and the PE never idles on a normalize.
 - V projection and the wo projection run as fp8(e4m3) DoubleRow matmuls
   (0.5 cycles/col, contraction 256/matmul): host pre-packs wv/wo in
   [128,2,cols] pair-block layout with per-column scales; hT gets an fp8
   copy (x16) on the ACT engine; Q/K/dots/AV/FFN stay bf16 (fp8 there
   fails the 2e-2 gate -- measured per-matmul on the reference).
   Runtime descales travel in a small "sc" constant tensor as
   per-partition scalar APs fused into scalar_tensor_tensor ops.
 - Attention AV uses pt (exp dots, keys on partitions) as the *stationary*
   operand so the output is [128 queries, 65]; softmax denominators are
   gathered strided, one batched reciprocal per pair, normalize fused
   (x rec x s_oall) into a single two-scalar tensor_scalar writing bf16;
   bf16 PE transposes restore [dh, token]; transposed tiles land directly
   in the fp8 pair-block oall operand of the wo DoubleRow matmul.
 - softmax exp is split across engines: 2/3 on ACT (AF.Exp), 1/3 on DVE
   via a Schraudolph bit-trick exp -- int16(x*2^7/ln2 + 127*2^7) written
   through a bitcast AP and read back as bf16 (~+-2% sawtooth, identical
   in numerator and denominator so softmax normalizes it away).
 - The attention j-loop is software-pipelined as in v2 (dots/exp lead,
   AV lags LAG slots, K for pair p+2 interleaved 2 matmuls/slot).
 - w1 is prefetched through the LN/attention phases (split across both
   SBUF stacks), wk is split so only pairs 0/1 occupy SBUF during LN,
   and 18 warmup matmuls during the initial DMA raise the PE p-state
   before the first real chain.
 - FFN1 is bf16; FFN2 runs entirely as fp8 weight-residual DoubleRow:
   w2 = hi+lo e4m3 pair-blocks sharing one per-column scale (~0.1% weight
   error, 2x fewer PE cycles than bf16), gelu writes g directly in
   unscaled e4m3, and the hi+lo chains accumulate into one psum descaled
   once per e-tile. Adds ~1.2e-2 error (activation quantization only).
 - FFN runs f-outer then e-outer so output tiles drain early.

Matmuls accumulate in fp32 PSUM. Cost-model time: 330.9us (baseline 420.2);
device rel err 1.6e-2 (gate 2e-2).
"""
import sys
sys.path.insert(0, "/opt/trn_rl_repo")

import numpy as np
import ml_dtypes

import concourse.bass as bass
import concourse.bass_isa as bass_isa
import concourse.tile as tile
from concourse import bacc, mybir

B, N, D = 2, 2048, 1024
H, DH = 16, 64
FF = 4096
NCORES = 8
T = N * B // NCORES          # 512 tokens per core
CPB = NCORES // B            # 4 cores per batch
ET = D // 128                # 8 embed tiles
FT = FF // 128               # 32 ffn tiles
NT = N // 128                # 16 key tiles per batch
NCH = N // 512               # 4 512-chunks per batch
HP = H // 2                  # 8 head pairs

dtb = mybir.dt.bfloat16
dtf = mybir.dt.float32
dt8 = mybir.dt.float8e4
AF = mybir.ActivationFunctionType
RED = bass_isa.ReduceOp
DR = mybir.MatmulPerfMode.DoubleRow
ALU = mybir.AluOpType
ts = bass.ts
H8S = 16.0                   # static scale for the fp8 copy of hT (h ~ N(0,1))


def build(stage_limit="E", zero_bias=False):
    nc = bacc.Bacc("TRN2", target_bir_lowering=False, debug=False)

    xbT_d = nc.dram_tensor("xbT", [D, N], dtb, kind="ExternalInput").ap()
    xoT_d = nc.dram_tensor("xoT", [D, T], dtb, kind="ExternalInput").ap()
    wq_d = nc.dram_tensor("wq", [D, D], dtb, kind="ExternalInput").ap()
    wk_d = nc.dram_tensor("wk", [D, D], dtb, kind="ExternalInput").ap()
    wv8_d = nc.dram_tensor("wv8", [128, ET * D], dt8, kind="ExternalInput").ap()
    wo8_d = nc.dram_tensor("wo8", [128, ET * D], dt8, kind="ExternalInput").ap()
    w1_d = nc.dram_tensor("w1", [D, FF], dtb, kind="ExternalInput").ap()
    # FFN2 as fp8 weight-residual (hi+lo DoubleRow chains, ~0.1% weight
    # error, one shared per-column scale, activation g in unscaled e4m3)
    w2h8_d = nc.dram_tensor("w2h8", [128, (FT // 2) * 2 * D], dt8,
                            kind="ExternalInput").ap()
    w2l8_d = nc.dram_tensor("w2l8", [128, (FT // 2) * 2 * D], dt8,
                            kind="ExternalInput").ap()
    bq_d = nc.dram_tensor("bq", [128, HP], dtf, kind="ExternalInput").ap()
    bk_d = nc.dram_tensor("bk", [128, HP], dtf, kind="ExternalInput").ap()
    bvw_d = nc.dram_tensor("bvw", [128, D], dtb, kind="ExternalInput").ap()
    b1e_d = nc.dram_tensor("b1e", [128, FT], dtf, kind="ExternalInput").ap()
    b2_d = nc.dram_tensor("b2", [128, ET], dtf, kind="ExternalInput").ap()
    id_d = nc.dram_tensor("ident", [128, 128], dtf, kind="ExternalInput").ap()
    idb_d = nc.dram_tensor("identb", [128, 128], dtb, kind="ExternalInput").ap()
    # sc: col0 = v descale 1/(H8S*s_wv); col1 = s_oall; cols 2..9 = wo
    # per-feature descale 1/(s_oall*s_wocol) per e-tile
    sc_d = nc.dram_tensor("sc", [128, 2 + 2 * ET], dtf, kind="ExternalInput").ap()
    outT_d = nc.dram_tensor("outT", [D, T], dtf, kind="ExternalOutput").ap()

    with tile.TileContext(nc) as tc:
        _body(nc, tc, xbT_d, xoT_d, wq_d, wk_d, wv8_d, wo8_d, w1_d,
              w2h8_d, w2l8_d,
              bq_d, bk_d, bvw_d, b1e_d, b2_d, id_d, idb_d, sc_d, outT_d,
              stage_limit, zero_bias)
    nc.finalize()
    return nc


def _body(nc, tc, xbT_d, xoT_d, wq_d, wk_d, wv8_d, wo8_d, w1_d,
          w2h8_d, w2l8_d,
          bq_d, bk_d, bvw_d, b1e_d, b2_d, id_d, idb_d, sc_d, outT_d,
          stage_limit, zero_bias):
    mm = nc.tensor.matmul

    def pool(name, bufs, space="SBUF", side=None):
        cm = tc.tile_pool(name=name, bufs=bufs, space=space, side=side)
        return cm, cm.__enter__()

    def close(*cms):
        for cm in cms:
            cm.__exit__(None, None, None)

    # ---------- persistent pools (right stack) ----------
    cpool_cm, cpool = pool("const", 1, side="right")
    w1a_cm, w1ap = pool("w1a", ET // 2, side="right")

    ones128 = cpool.tile([128, 128], dtb)
    nc.vector.memset(ones128[:], 1.0)
    eps128 = cpool.tile([128, 1], dtf)
    nc.vector.memset(eps128[:], 1e-5)
    ident = cpool.tile([128, 128], dtf)
    identb = cpool.tile([128, 128], dtb)
    bq_s = cpool.tile([128, HP], dtf)
    bk_s = cpool.tile([128, HP], dtf)
    bvw_s = cpool.tile([128, D], dtb)
    b1e_s = cpool.tile([128, FT], dtf)
    b2_s = cpool.tile([128, ET], dtf)
    sc_s = cpool.tile([128, 2 + 2 * ET], dtf)
    # (const DMAs are emitted after the startup-critical x/wq loads)

    # PE p-state warmup: ~3us of dummy matmuls during the initial DMA wait
    # so the first real chains run at full clock.
    warm_cm, warmp = pool("warm", 1, space="PSUM")
    wps = warmp.tile([128, 128], dtf)
    for i in range(18):
        mm(wps[:], ones128[:], ones128[:], start=(i == 0), stop=(i == 17))
    close(warm_cm)

    # ---------- left stack: pools living into the attention phase ----------
    # (wo8/oall sit at the bottom so they can outlive hT..kt: LIFO closes)
    wo8_cm, wo8p = pool("wo8", ET // 2)
    oall_cm, oallp = pool("oall", HP // 2)
    hT_cm, hTp = pool("hT", ET)
    v_cm, vp = pool("v", NT)
    qt_cm, qtp = pool("qt", HP)
    wk_cm, wkp = pool("wk", ET)
    kt_cm, ktp = pool("kt", 3)

    # LN-phase pools (wq last/topmost: it frees right after chunk 0's Q)
    wv8_cm, wv8p = pool("wv8", ET // 2)
    h8_cm, h8p = pool("h8", ET)
    xb_cm, xbp = pool("xb", 14)
    sq_cm, sqp = pool("sq", ET + 4)
    ab_cm, abp = pool("ab", 1)
    sps_cm, spsp = pool("sps", 3, space="PSUM")
    qps_cm, qpsp = pool("qps", 2, space="PSUM")
    vps_cm, vpsp = pool("vps", 2, space="PSUM")
    kps0_cm, kps0p = pool("kps0", 1, space="PSUM")
    wq_cm, wqp = pool("wq", ET)

    # DMA order = emission order (single queue): x chunk 0 first, then wq/wv8
    # (needed early), then the rest of x, then wk, wo8, w1 (prefetched
    # through the attention phase).
    def load_xchunk(c):
        out = []
        for e in range(ET):
            t = xbp.tile([128, 512], dtb, tag="xb", name=f"xb{c}_{e}")
            nc.sync.dma_start(out=t[:], in_=xbT_d[ts(e, 128), ts(c, 512)])
            out.append(t)
        return out

    xbc = {0: load_xchunk(0)}
    wq_sb, wv8_sb = [], []
    for e in range(ET):
        tq = wqp.tile([128, D], dtb, tag="wq")
        nc.sync.dma_start(out=tq[:], in_=wq_d[ts(e, 128), :])
        wq_sb.append(tq)
    for i in range(ET // 2):
        tv = wv8p.tile([128, 2, D], dt8, tag="wv8")
        nc.sync.dma_start(out=tv[:], in_=wv8_d[:, ts(i, 2 * D)])
        wv8_sb.append(tv)
    for t_, d_ in ((bq_s, bq_d), (bk_s, bk_d), (bvw_s, bvw_d),
                   (ident, id_d), (identb, idb_d), (b1e_s, b1e_d),
                   (b2_s, b2_d), (sc_s, sc_d)):
        nc.sync.dma_start(out=t_[:], in_=d_[:, :])
    for c in range(1, NCH):
        xbc[c] = load_xchunk(c)
    wk_sb = []
    for e in range(ET):
        tk = wkp.tile([128, 256], dtb, tag="wk")
        nc.sync.dma_start(out=tk[:], in_=wk_d[ts(e, 128), 0:256])
        wk_sb.append(tk)
    wo8_sb = []
    for i in range(ET // 2):
        tw = wo8p.tile([128, 2, D], dt8, tag="wo8")
        nc.sync.dma_start(out=tw[:], in_=wo8_d[:, ts(i, 2 * D)])
        wo8_sb.append(tw)
    w1_sb = []
    for e in range(ET // 2):
        tw1 = w1ap.tile([128, FF], dtb, tag="w1a")
        nc.sync.dma_start(out=tw1[:], in_=w1_d[ts(e, 128), :])
        w1_sb.append(tw1)

    hT = [hTp.tile([128, N], dtb, tag="hT", name=f"hT{e}") for e in range(ET)]
    qt = [qtp.tile([128, T], dtb, tag="qt", name=f"qt{p}") for p in range(HP)]
    v_sb = [vp.tile([128, H * (DH + 1)], dtb, tag="v", name=f"v{j}")
            for j in range(NT)]
    kt_tiles = {}

    def alloc_kt(p):
        kt_tiles[p] = ktp.tile([128, N], dtb, tag="kt", name=f"kt{p}")

    alloc_kt(0)
    alloc_kt(1)

    # ============ stage A+B: per-chunk LN1 -> Q(c0) / V(c) / K0(c) ========
    def ln_normalize(pl, x_slices, S_ps, SQr, out_slices, cw):
        """A = rsqrt(var+eps), B = mean*A; out = x*A - B (all [128, cw])."""
        mean = pl.tile([128, cw], dtf, tag="ab_mean")
        var = pl.tile([128, cw], dtf, tag="ab_var")
        m2 = pl.tile([128, cw], dtf, tag="ab_m2")
        Ar = pl.tile([128, cw], dtf, tag="ab_A")
        Acb = pl.tile([128, cw], dtb, tag="ab_Acb")
        Bcb = pl.tile([128, cw], dtb, tag="ab_Bcb")
        nc.vector.tensor_scalar_mul(mean[:], S_ps[:], 1.0 / D)
        nc.vector.tensor_mul(m2[:], mean[:], mean[:])
        nc.vector.scalar_tensor_tensor(var[:], SQr[:], 1.0 / D, m2[:],
                                       ALU.mult, ALU.subtract)
        nc.scalar.activation(var[:], var[:], AF.Sqrt, bias=eps128[:])
        nc.vector.reciprocal(Ar[:], var[:])
        nc.vector.tensor_mul(Bcb[:], mean[:], Ar[:])   # B = mean*A
        nc.vector.tensor_copy(Acb[:], Ar[:])
        for xsl, osl in zip(x_slices, out_slices):
            nc.vector.tensor_mul(osl, xsl, Acb[:])
            nc.vector.tensor_sub(osl, osl, Bcb[:])

    def squares(sql, x_slices, cw):
        sq = []
        for xsl in x_slices:
            t = sql.tile([128, cw], dtb, tag="sq")
            nc.vector.tensor_mul(t[:], xsl, xsl)
            sq.append(t)
        return sq

    def pe_sum(psp, tag, slices, cw):
        """sum over tiles via all-ones stationary matmul chain (replicated)."""
        s = psp.tile([128, cw], dtf, tag=tag)
        for i, sl in enumerate(slices):
            mm(s[:], ones128[:], sl, start=(i == 0), stop=(i == len(slices) - 1))
        return s

    def wk_slice(e, p):
        if p < 2:
            return wk_sb[e][:, ts(p, 128)]
        return wkB_sb[e][:, ts(p - 2, 128)]

    def k_chain(kt_t, p, c, ps_pool, ps_tag):
        k_ps = ps_pool.tile([128, 512], dtf, tag=ps_tag)
        for e in range(ET):
            mm(k_ps[:], wk_slice(e, p), hT[e][:, ts(c, 512)],
               start=(e == 0), stop=(e == ET - 1))
        if p < 2:
            # LN phase: ACT is congested with h8/V copies and kt0's last
            # bias gates the attention start; DVE has slack here
            nc.vector.tensor_scalar_add(kt_t[:, ts(c, 512)], k_ps[:],
                                        bk_s[:, p:p + 1])
        else:
            nc.scalar.activation(kt_t[:, ts(c, 512)], k_ps[:], AF.Identity,
                                 bias=bk_s[:, p:p + 1])

    def emit_qchain(p):
        q_ps = qpsp.tile([128, T], dtf, tag="qps")
        for e in range(ET):
            mm(q_ps[:], wq_sb[e][:, ts(p, 128)], hT[e][:, 0:T],
               start=(e == 0), stop=(e == ET - 1))
        nc.scalar.activation(qt[p][:], q_ps[:], AF.Identity,
                             bias=bq_s[:, p:p + 1])

    def emit_work(c, h8c):
        """matmul work for chunk c (emitted once chunk c's h is ready):
        Q pairs {2c, 2c+1}, K pairs 0/1, V (fp8 DoubleRow)."""
        emit_qchain(2 * c)
        k_chain(kt_tiles[0], 0, c, kps0p, "kps0")
        emit_qchain(2 * c + 1)
        k_chain(kt_tiles[1], 1, c, kps0p, "kps0")
        for j in range(4 * c, 4 * c + 4):
            vt = v_sb[j]
            v3 = vt[:].rearrange("p (h c) -> p h c", c=DH + 1)
            nc.vector.memset(v3[:, :, DH:DH + 1], 1.0)
            for c2 in range(2):
                v_ps = vpsp.tile([128, 512], dtf, tag="vps")
                for i in range(ET // 2):
                    mm(v_ps[:], h8c[i][:, :, ts(j - 4 * c, 128)],
                       wv8_sb[i][:, :, ts(c2, 512)],
                       start=(i == 0), stop=(i == ET // 2 - 1),
                       perf_mode=DR)
                if zero_bias:
                    # bv == 0: plain descaled copy on the (idle) ACT engine
                    nc.scalar.activation(
                        v3[:, c2 * 8:(c2 + 1) * 8, 0:DH],
                        v_ps[:].rearrange("p (h c) -> p h c", c=DH),
                        AF.Copy, scale=sc_s[:, 0:1])
                else:
                    bsl = bvw_s[:, ts(c2, 512)].rearrange(
                        "p (h c) -> p h c", c=DH)
                    nc.vector.scalar_tensor_tensor(
                        v3[:, c2 * 8:(c2 + 1) * 8, 0:DH],
                        v_ps[:].rearrange("p (h c) -> p h c", c=DH),
                        sc_s[:, 0:1], bsl, ALU.mult, ALU.add)

    # software-pipelined chunk loop: stats of chunk c+1 go into the PE queue
    # BEFORE the matmul work of chunk c, so the (in-order) PE never sits
    # behind a wait for chunk c's normalize.
    prev = None
    for c in range(NCH):
        csl = ts(c, 512)
        xc = [xbc[c][e][:, :] for e in range(ET)]
        S_ps = pe_sum(spsp, "S", xc, 512)
        sq = squares(sqp, xc, 512)
        SQr = pe_sum(spsp, "S", [t[:] for t in sq], 512)
        if prev is not None:
            emit_work(*prev)
        ln_normalize(abp, xc, S_ps, SQr,
                     [hT[e][:, csl] for e in range(ET)], 512)
        # fp8 copy of h (x H8S) for the DoubleRow V projection, on ACT
        h8c = [h8p.tile([128, 2, 512], dt8, tag="h8", name=f"h8_{c}_{i}")
               for i in range(ET // 2)]
        for i in range(ET // 2):
            for jb in range(2):
                nc.scalar.activation(h8c[i][:, jb, :],
                                     hT[2 * i + jb][:, csl],
                                     AF.Copy, scale=H8S)
        prev = (c, h8c)
    emit_work(*prev)
    close(wq_cm)

    close(kps0_cm, vps_cm, qps_cm, sps_cm, ab_cm, sq_cm,
          xb_cm, h8_cm, wv8_cm)
    if stage_limit == "A":
        close(kt_cm, wk_cm, qt_cm, v_cm, hT_cm, oall_cm, wo8_cm,
              w1a_cm, cpool_cm)
        return

    # wk columns for pairs 2-7 (left stack, on top of kt: closes first).
    wkB_cm, wkBp = pool("wkB", ET)
    wkB_sb = []
    for e in range(ET):
        tk = wkBp.tile([128, 768], dtb, tag="wkB")
        nc.sync.dma_start(out=tk[:], in_=wk_d[ts(e, 128), 256:D])
        wkB_sb.append(tk)

    # second half of w1 + residual prefetch (right stack, closed at the very
    # end). DMAs queue behind w1a and land during the attention phase.
    w1b_cm, w1bp = pool("w1b", ET // 2, side="right")
    for e in range(ET // 2, ET):
        tw1 = w1bp.tile([128, FF], dtb, tag="w1b")
        nc.sync.dma_start(out=tw1[:], in_=w1_d[ts(e, 128), :])
        w1_sb.append(tw1)
    xo_cm, xop = pool("xo", ET, side="right")
    xo_sb = []
    for e in range(ET):
        tx = xop.tile([128, T], dtb, tag="xo")
        nc.sync.dma_start(out=tx[:], in_=xoT_d[ts(e, 128), :])
        xo_sb.append(tx)

    # ============ stage C: attention, software-pipelined ============
    # Per pair p's j-loop: dots/exp lead, AV lags LAG slots (so pair p-1's
    # oT drains before AV(p,0) needs its PSUM slot), K chains for pair p+2
    # fill PE gaps, and pair p-1's transposes ride the first 8 slots.
    LAG = 6
    pt_cm, ptp = pool("pt", LAG + 3)
    onr_cm, onrp = pool("onr", 12)
    rec_cm, recp = pool("rec", 4)
    dps_cm, dpsp = pool("dps", 2, space="PSUM")
    ops_cm, opsp = pool("ops", 1, space="PSUM")
    kps_cm, kpsp = pool("kps", 1, space="PSUM")
    trp_cm, trpp = pool("trp", 1, space="PSUM")

    oall8 = [oallp.tile([128, 2, T], dt8, tag="oall", name=f"oall{i}")
             for i in range(HP // 2)]

    def emit_av(oT, p, j, pt):
        for h2 in range(2):
            voff = (2 * p + h2) * (DH + 1)
            # one accumulation group per 2KB zero region (bank): start
            # zeroes the whole bank, so the 4 qc-chains share one group
            for qc in range(4):
                mm(oT[:, h2 * 512 + qc * 65: h2 * 512 + qc * 65 + 65],
                   pt[:, h2 * T + qc * 128: h2 * T + (qc + 1) * 128],
                   v_sb[j][:, voff: voff + DH + 1],
                   start=(j == 0 and qc == 0),
                   stop=(j == NT - 1 and qc == 3))

    def emit_norm(oT, p):
        """batched reciprocal + scale for the 8 (head, qchunk) outputs of
        pair p; returns normalized bf16 [128, DH] tiles (x s_oall), which
        get transposed during the next pair."""
        den = recp.tile([128, 8], dtf, tag="den")
        for h2 in range(2):
            dsl = oT[:, h2 * 512: h2 * 512 + 260].rearrange(
                "p (q s) -> p q s", s=DH + 1)[:, :, DH:DH + 1]
            nc.vector.tensor_copy(
                den[:, h2 * 4:(h2 + 1) * 4].rearrange("p (q s) -> p q s", s=1),
                dsl)
        rec = recp.tile([128, 8], dtf, tag="rec")
        nc.vector.reciprocal(rec[:], den[:])
        out = []
        for i in range(8):
            h2, qc = divmod(i, 4)
            base = h2 * 512 + qc * 65
            onr = onrp.tile([128, DH], dtb, tag="onr", name=f"onr{p}_{i}")
            nc.vector.tensor_scalar(onr[:], oT[:, base: base + DH],
                                    rec[:, i:i + 1], sc_s[:, 1:2],
                                    ALU.mult, ALU.mult)
            out.append(onr)
        return out

    def emit_transpose(p, i, onr):
        h2, qc = divmod(i, 4)
        tr = trpp.tile([64, 128], dtb, tag="tr")
        nc.tensor.transpose(tr[:], onr[:], identb[:])
        nc.vector.tensor_copy(
            oall8[p // 2][h2 * DH:(h2 + 1) * DH, p % 2, ts(qc, 128)], tr[:])

    prev_norm = None
    for p in range(HP):
        k_items = []
        if p + 2 < HP:
            alloc_kt(p + 2)
            k_items = [(c, e) for c in range(NCH) for e in range(ET)]
        kt_cur = kt_tiles[p]
        k_ps = None
        oT = opsp.tile([128, 1024], dtf, tag="oT")
        ptq = {}
        for j in range(NT):
            dp = dpsp.tile([128, 2 * T], dtf, tag="dp")
            mm(dp[:, 0:T], kt_cur[0:64, ts(j, 128)], qt[p][0:64, :],
               start=True, stop=True)
            mm(dp[:, T:2 * T], kt_cur[64:128, ts(j, 128)], qt[p][64:128, :],
               start=True, stop=True)
            pt = ptp.tile([128, 2 * T], dtb, tag="pt")
            if j % 3 == 2:
                # Schraudolph exp on DVE: int16(x*2^7/ln2 + 127*2^7) bits
                # read back as bf16 ~= exp(x) (+-2% sawtooth; consistent
                # between numerator and denominator, so softmax cancels it)
                nc.vector.tensor_scalar(pt[:].bitcast(mybir.dt.int16), dp[:],
                                        184.6617, 16249.6,
                                        ALU.mult, ALU.add)
            else:
                nc.scalar.activation(pt[:], dp[:], AF.Exp)
            ptq[j] = pt
            if prev_norm is not None and j < 8:
                emit_transpose(p - 1, j, prev_norm[j])
            if j >= LAG:
                emit_av(oT, p, j - LAG, ptq.pop(j - LAG))
            # interleave 2 K-chain matmuls for pair p+2
            for _ in range(2):
                if not k_items:
                    continue
                c, e = k_items.pop(0)
                if e == 0:
                    k_ps = kpsp.tile([128, 512], dtf, tag="kps")
                mm(k_ps[:], wk_slice(e, p + 2),
                   hT[e][:, ts(c, 512)],
                   start=(e == 0), stop=(e == ET - 1))
                if e == ET - 1:
                    nc.vector.tensor_scalar_add(
                        kt_tiles[p + 2][:, ts(c, 512)], k_ps[:],
                        bk_s[:, p + 2:p + 3])
        for j in range(NT - LAG, NT):
            emit_av(oT, p, j, ptq.pop(j))
        prev_norm = emit_norm(oT, p)
    for i in range(8):
        emit_transpose(HP - 1, i, prev_norm[i])

    close(trp_cm, kps_cm, ops_cm, dps_cm, rec_cm, onr_cm, pt_cm)
    close(wkB_cm, kt_cm, wk_cm, qt_cm, v_cm, hT_cm)
    if stage_limit == "C":
        close(oall_cm, wo8_cm, xo_cm, w1b_cm, w1a_cm, cpool_cm)
        return

    # ============ stage D: wo proj (fp8 DR) + residual + LN2 ============
    x2_cm, x2p = pool("x2", ET, side="right")
    h2_cm, h2p = pool("h2", ET, side="right")
    x2b_cm, x2bp = pool("x2b", ET)
    sqd_cm, sqdp = pool("sqd", ET)
    abd_cm, abdp = pool("abd", 1)
    prs_cm, prsp = pool("prs", 2, space="PSUM")
    s2s_cm, s2sp = pool("s2s", 1, space="PSUM")
    sq2s_cm, sq2sp = pool("sq2s", 1, space="PSUM")

    x2, x2b = [], []
    S2_ps = s2sp.tile([128, T], dtf, tag="S2")
    for e in range(ET):
        pr_ps = prsp.tile([128, T], dtf, tag="prs")
        for i in range(ET // 2):
            mm(pr_ps[:], wo8_sb[i][:, :, ts(e, 128)], oall8[i][:, :, :],
               start=(i == 0), stop=(i == ET // 2 - 1), perf_mode=DR)
        tx2 = x2p.tile([128, T], dtf, tag="x2")
        nc.vector.scalar_tensor_tensor(tx2[:], pr_ps[:], sc_s[:, 2 + e:3 + e],
                                       xo_sb[e][:], ALU.mult, ALU.add)
        x2.append(tx2)
        tb = x2bp.tile([128, T], dtb, tag="x2b")
        nc.scalar.activation(tb[:], tx2[:], AF.Copy)
        x2b.append(tb)
        mm(S2_ps[:], ones128[:], tb[:], start=(e == 0), stop=(e == ET - 1))

    h2 = [h2p.tile([128, T], dtb, tag="h2", name=f"h2_{e}")
          for e in range(ET)]
    sq2 = squares(sqdp, [t[:, :] for t in x2b], T)
    SQ2r = pe_sum(sq2sp, "SQ2", [t[:] for t in sq2], T)
    ln_normalize(abdp, [t[:, :] for t in x2b], S2_ps, SQ2r,
                 [t[:, :] for t in h2], T)
    close(sq2s_cm, s2s_cm, prs_cm, abd_cm, sqd_cm, x2b_cm)
    close(oall_cm, wo8_cm)
    if stage_limit == "D":
        close(h2_cm, x2_cm, xo_cm, w1b_cm, w1a_cm, cpool_cm)
        return

    # ============ stage E: FFN ============
    # FFN1 bf16; FFN2 entirely as fp8 weight-residual DoubleRow: hi and lo
    # chains share one per-column scale (psum descales once per e-tile),
    # gelu writes g directly in unscaled e4m3.
    FP = FT // 2                         # 16 f-pair tiles
    g8_cm, g8p = pool("g8", FP)
    w2h_cm, w2hp = pool("w2h", FP)
    w2l_cm, w2lp = pool("w2l", FP)
    w2h_sb, w2l_sb = [], []
    for i in range(FP):
        th = w2hp.tile([128, 2, D], dt8, tag="w2h")
        nc.sync.dma_start(out=th[:], in_=w2h8_d[:, ts(i, 2 * D)])
        w2h_sb.append(th)
    for i in range(FP):
        tl = w2lp.tile([128, 2, D], dt8, tag="w2l")
        nc.sync.dma_start(out=tl[:], in_=w2l8_d[:, ts(i, 2 * D)])
        w2l_sb.append(tl)
    aps_cm, apsp = pool("aps", 3, space="PSUM")
    g8_sb = [g8p.tile([128, 2, T], dt8, tag="g8", name=f"g8_{i}")
             for i in range(FP)]
    for f in range(FT):
        a_ps = apsp.tile([128, T], dtf, tag="aps")
        for e in range(ET):
            mm(a_ps[:], w1_sb[e][:, ts(f, 128)], h2[e][:],
               start=(e == 0), stop=(e == ET - 1))
        nc.scalar.activation(g8_sb[f // 2][:, f % 2, :], a_ps[:],
                             AF.Gelu, bias=b1e_s[:, f:f + 1])
    close(aps_cm)

    ob_cm, obp = pool("ob", 4)
    yps_cm, ypsp = pool("yps", 3, space="PSUM")
    for e in range(ET):
        y_ps = ypsp.tile([128, T], dtf, tag="yps")
        for i in range(FP):
            mm(y_ps[:], w2h_sb[i][:, :, ts(e, 128)], g8_sb[i][:, :, :],
               start=(i == 0), stop=False, perf_mode=DR)
        for i in range(FP):
            mm(y_ps[:], w2l_sb[i][:, :, ts(e, 128)], g8_sb[i][:, :, :],
               start=False, stop=(i == FP - 1), perf_mode=DR)
        ob = obp.tile([128, T], dtf, tag="ob")
        nc.vector.scalar_tensor_tensor(ob[:], y_ps[:],
                                       sc_s[:, 2 + ET + e:3 + ET + e],
                                       x2[e][:], ALU.mult, ALU.add)
        if not zero_bias:
            nc.vector.tensor_scalar_add(ob[:], ob[:], b2_s[:, e:e + 1])
        nc.sync.dma_start(out=outT_d[ts(e, 128), :], in_=ob[:])
    close(yps_cm, ob_cm, w2l_cm, w2h_cm, g8_cm)

    close(h2_cm, x2_cm, xo_cm, w1b_cm, w1a_cm, cpool_cm)


_NC_CACHE = {}


def _zero_bias_flag(ln1_b):
    return bool(np.all(np.asarray(ln1_b) == 0.0))


def _get_nc(zero_bias=False):
    key = ("nc", zero_bias)
    if key not in _NC_CACHE:
        _NC_CACHE[key] = build(zero_bias=zero_bias)
    return _NC_CACHE[key]


def _vec_tiles(v, ntiles):
    return np.ascontiguousarray(
        np.asarray(v, np.float32).reshape(ntiles, 128).T)


def _pair_blocks(w8):
    """[K, cols] quantized array -> [128, (K//256)*2*cols] pair-block layout."""
    blocks = []
    for i in range(w8.shape[0] // 256):
        for j in range(2):
            blocks.append(w8[(2 * i + j) * 128:(2 * i + j + 1) * 128, :])
    return np.ascontiguousarray(np.concatenate(blocks, axis=1))


def _fp8_pairs(w, colscale):
    """[D, D] fp32 -> [128, (D//256)*2*D] e4m3 pair-block layout."""
    return _pair_blocks((w * colscale).astype(ml_dtypes.float8_e4m3))


def prepare_in_maps(x, wq, wk, wv, wo, w1, b1, w2, b2,
                    ln1_g, ln1_b, ln2_g, ln2_b):
    bf = ml_dtypes.bfloat16
    f32 = np.float32
    x = np.asarray(x, f32)
    wq = np.asarray(wq, f32); wk = np.asarray(wk, f32)
    wv = np.asarray(wv, f32); w1 = np.asarray(w1, f32)
    wo = np.asarray(wo, f32)
    g1 = np.asarray(ln1_g, f32)[:, None]
    b1v = np.asarray(ln1_b, f32)
    g2 = np.asarray(ln2_g, f32)[:, None]
    b2v = np.asarray(ln2_b, f32)
    bq = (b1v @ wq).astype(f32)          # [D] per-output-col constants
    bk = (b1v @ wk).astype(f32)
    bv = (b1v @ wv).astype(f32)
    b1eff = (np.asarray(b1, f32) + b2v @ w1).astype(f32)

    wv_g = wv * g1
    s_wv = 120.0 / max(1e-30, np.abs(wv_g).max())
    # bound on |attn out| <= max |v| row; scale so fp8 oall stays in range
    vbound = 6.0 * np.linalg.norm(wv_g, axis=0).max() + np.abs(bv).max()
    s_oall = 120.0 / max(1e-30, vbound)
    s_wocol = 120.0 / np.maximum(np.abs(wo).max(axis=0), 1e-30)
    # FFN2 split: rows 0..FF/2 as fp8 hi+lo residual, rest bf16 pre-scaled
    # by the shared per-column scale s2col (g stays in unscaled e4m3)
    w2f = np.asarray(w2, f32)
    s2col = 120.0 / np.maximum(np.abs(w2f).max(axis=0), 1e-30)
    f8t = ml_dtypes.float8_e4m3
    w2s = w2f * s2col[None, :]
    w2hi = np.asarray(w2s, f8t)
    w2lo = np.asarray(w2s - w2hi.astype(f32), f8t)
    sc = np.zeros((128, 2 + 2 * ET), f32)
    sc[:, 0] = 1.0 / (H8S * s_wv)
    sc[:, 1] = s_oall
    for e in range(ET):
        sc[:, 2 + e] = 1.0 / (s_oall * s_wocol[e * 128:(e + 1) * 128])
        sc[:, 2 + ET + e] = 1.0 / s2col[e * 128:(e + 1) * 128]

    shared = {
        "wq": np.ascontiguousarray((wq * g1).astype(bf)),
        "wk": np.ascontiguousarray((wk * g1).astype(bf)),
        "wv8": _fp8_pairs(wv_g, s_wv),
        "wo8": _fp8_pairs(wo, s_wocol[None, :]),
        "w1": np.ascontiguousarray((w1 * g2).astype(bf)),
        "w2h8": _pair_blocks(w2hi),
        "w2l8": _pair_blocks(w2lo),
        "bq": np.ascontiguousarray(bq.reshape(HP, 128).T),
        "bk": np.ascontiguousarray(bk.reshape(HP, 128).T),
        "bvw": np.ascontiguousarray(np.tile(bv.astype(bf), (128, 1))),
        "b1e": _vec_tiles(b1eff, FT),
        "b2": _vec_tiles(b2, ET),
        "ident": np.ascontiguousarray(np.eye(128, dtype=f32)),
        "identb": np.ascontiguousarray(np.eye(128, dtype=bf)),
        "sc": np.ascontiguousarray(sc),
    }
    in_maps = []
    for c in range(NCORES):
        b, s = divmod(c, CPB)
        rot = np.concatenate([x[b, s * T:], x[b, :s * T]], axis=0)  # own first
        m = dict(shared)
        m["xbT"] = np.ascontiguousarray(rot.T.astype(bf))
        m["xoT"] = np.ascontiguousarray(x[b, s * T:(s + 1) * T].T.astype(bf))
        in_maps.append(m)
    return in_maps


def assemble_output(results):
    out = np.empty((B, N, D), np.float32)
    for c in range(NCORES):
        b, s = divmod(c, CPB)
        out[b, s * T:(s + 1) * T, :] = results[c]["outT"].T
    return out


def kernel(x, wq, wk, wv, wo, w1, b1, w2, b2, ln1_g, ln1_b, ln2_g, ln2_b):
    from concourse.bass_utils import run_bass_kernel_spmd

    nc = _get_nc(_zero_bias_flag(ln1_b))
    in_maps = prepare_in_maps(x, wq, wk, wv, wo, w1, b1, w2, b2,
                              ln1_g, ln1_b, ln2_g, ln2_b)
    res = run_bass_kernel_spmd(nc, in_maps, core_ids=list(range(NCORES)))
    return assemble_output(res.results)



# revision 100
# speedup vs baseline: 1.0213x; 1.0213x over previous
"""Trainium2 Bass kernel for a pre-norm transformer encoder layer.

Problem: x[2,2048,1024]; LN1 -> QKV (16 heads x 64) -> softmax(QK^T) V
-> wo -> +res -> LN2 -> GELU(h@w1+b1)@w2+b2 -> +res.

Sharding: token-parallel over B*N = 4096 tokens; each of the 8 cores owns
512 tokens (cores 0-3: batch 0, cores 4-7: batch 1). Each core recomputes
K/V for its whole batch (no collectives). All activations are kept in
transposed layout [feature, token] so every matmul contracts over the
partition dim. Host pre-rotates each core's batch so its own 512 tokens
are always columns 0:512 -> one NEFF shared by all 8 cores.

v3 restructure (fp8-DoubleRow on the error-tolerant matmuls, multi-engine
softmax, latency-pipelined phases):
 - LN gammas are folded into wq/wk/wv/w1 rows on the host; LN betas become
   per-output-feature constants (b@W) applied for free as per-partition
   bias APs in the PSUM->SBUF copies. Device LN is (x-mu)*rstd = x*A - B.
 - LN statistics (sum, sum-of-squares) both via all-ones [128,128]
   stationary PE matmul chains (replicated output); the chunk loop is
   software-pipelined so chunk c+1's stats sit in the (in-order) PE queue
   before chunk c's Q/K/V work and the PE never idles on a normalize.
 - V projection and the wo projection run as fp8(e4m3) DoubleRow matmuls
   (0.5 cycles/col, contraction 256/matmul): host pre-packs wv/wo in
   [128,2,cols] pair-block layout with per-column scales; hT gets an fp8
   copy (x16) on the ACT engine; Q/K/dots/AV/FFN stay bf16 (fp8 there
   fails the 2e-2 gate -- measured per-matmul on the reference).
   Runtime descales travel in a small "sc" constant tensor as
   per-partition scalar APs fused into scalar_tensor_tensor ops.
 - Attention AV uses pt (exp dots, keys on partitions) as the *stationary*
   operand so the output is [128 queries, 65]; softmax denominators are
   gathered strided, one batched reciprocal per pair, normalize fused
   (x rec x s_oall) into a single two-scalar tensor_scalar writing bf16;
   bf16 PE transposes restore [dh, token]; transposed tiles land directly
   in the fp8 pair-block oall operand of the wo DoubleRow matmul.
 - softmax exp is split across engines: 2/3 on ACT (AF.Exp), 1/3 on DVE
   via a Schraudolph bit-trick exp -- int16(x*2^7/ln2 + 127*2^7) written
   through a bitcast AP and read back as bf16 (~+-2% sawtooth, identical
   in numerator and denominator so softmax normalizes it away).
 - The attention j-loop is software-pipelined as in v2 (dots/exp lead,
   AV lags LAG slots, K for pair p+2 interleaved 2 matmuls/slot).
 - w1 is prefetched through the LN/attention phases (split across both
   SBUF stacks), wk is split so only pairs 0/1 occupy SBUF during LN,
   and 18 warmup matmuls during the initial DMA raise the PE p-state
   before the first real chain.
 - FFN1 is bf16; FFN2 runs entirely as fp8 weight-residual DoubleRow:
   w2 = hi+lo e4m3 pair-blocks sharing one per-column scale (~0.1% weight
   error, 2x fewer PE cycles than bf16), gelu writes g directly in
   unscaled e4m3, and the hi+lo chains accumulate into one psum descaled
   once per e-tile. Adds ~1.2e-2 error (activation quantization only).
 - FFN runs f-outer then e-outer so output tiles drain early.

Matmuls accumulate in fp32 PSUM. Cost-model time: 330.9us (baseline 420.2);
device rel err 1.6e-2 (gate 2e-2).
"""
import sys
sys.path.insert(0, "/opt/trn_rl_repo")

import numpy as np
import ml_dtypes

import concourse.bass as bass
import concourse.bass_isa as bass_isa
import concourse.tile as tile
from concourse import bacc, mybir

B, N, D = 2, 2048, 1024
H, DH = 16, 64
FF = 4096
NCORES = 8
T = N * B // NCORES          # 512 tokens per core
CPB = NCORES // B            # 4 cores per batch
ET = D // 128                # 8 embed tiles
FT = FF // 128               # 32 ffn tiles
NT = N // 128                # 16 key tiles per batch
NCH = N // 512               # 4 512-chunks per batch
HP = H // 2                  # 8 head pairs

dtb = mybir.dt.bfloat16
dtf = mybir.dt.float32
dt8 = mybir.dt.float8e4
AF = mybir.ActivationFunctionType
RED = bass_isa.ReduceOp
DR = mybir.MatmulPerfMode.DoubleRow
ALU = mybir.AluOpType
ts = bass.ts
H8S = 16.0                   # static scale for the fp8 copy of hT (h ~ N(0,1))


def build(stage_limit="E", zero_bias=False):
    nc = bacc.Bacc("TRN2", target_bir_lowering=False, debug=False)

    xbT_d = nc.dram_tensor("xbT", [D, N], dtb, kind="ExternalInput").ap()
    xoT_d = nc.dram_tensor("xoT", [D, T], dtb, kind="ExternalInput").ap()
    wq_d = nc.dram_tensor("wq", [D, D], dtb, kind="ExternalInput").ap()
    wk_d = nc.dram_tensor("wk", [D, D], dtb, kind="ExternalInput").ap()
    wv8_d = nc.dram_tensor("wv8", [128, ET * D], dt8, kind="ExternalInput").ap()
    wo8_d = nc.dram_tensor("wo8", [128, ET * D], dt8, kind="ExternalInput").ap()
    w1_d = nc.dram_tensor("w1", [D, FF - FF // 4], dtb,
                          kind="ExternalInput").ap()
    w1h8_d = nc.dram_tensor("w1h8", [128, (ET // 2) * 2 * (FF // 4)], dt8,
                            kind="ExternalInput").ap()
    w1l8_d = nc.dram_tensor("w1l8", [128, (ET // 2) * 2 * (FF // 4)], dt8,
                            kind="ExternalInput").ap()
    # FFN2 as fp8 weight-residual (hi+lo DoubleRow chains, ~0.1% weight
    # error, one shared per-column scale, activation g in unscaled e4m3)
    w2h8_d = nc.dram_tensor("w2h8", [128, (FT // 2) * 2 * D], dt8,
                            kind="ExternalInput").ap()
    w2l8_d = nc.dram_tensor("w2l8", [128, (FT // 2) * 2 * D], dt8,
                            kind="ExternalInput").ap()
    bq_d = nc.dram_tensor("bq", [128, HP], dtf, kind="ExternalInput").ap()
    bk_d = nc.dram_tensor("bk", [128, HP], dtf, kind="ExternalInput").ap()
    bvw_d = nc.dram_tensor("bvw", [128, D], dtb, kind="ExternalInput").ap()
    b1e_d = nc.dram_tensor("b1e", [128, FT], dtf, kind="ExternalInput").ap()
    b2_d = nc.dram_tensor("b2", [128, ET], dtf, kind="ExternalInput").ap()
    id_d = nc.dram_tensor("ident", [128, 128], dtf, kind="ExternalInput").ap()
    idb_d = nc.dram_tensor("identb", [128, 128], dtb, kind="ExternalInput").ap()
    # sc: col0 = v descale 1/(H8S*s_wv); col1 = s_oall; cols 2..9 = wo
    # per-feature descale 1/(s_oall*s_wocol) per e-tile
    sc_d = nc.dram_tensor("sc", [128, 2 + 2 * ET + FT // 4], dtf, kind="ExternalInput").ap()
    outT_d = nc.dram_tensor("outT", [D, T], dtf, kind="ExternalOutput").ap()

    with tile.TileContext(nc) as tc:
        _body(nc, tc, xbT_d, xoT_d, wq_d, wk_d, wv8_d, wo8_d, w1_d,
              w1h8_d, w1l8_d, w2h8_d, w2l8_d,
              bq_d, bk_d, bvw_d, b1e_d, b2_d, id_d, idb_d, sc_d, outT_d,
              stage_limit, zero_bias)
    nc.finalize()
    return nc


def _body(nc, tc, xbT_d, xoT_d, wq_d, wk_d, wv8_d, wo8_d, w1_d,
          w1h8_d, w1l8_d, w2h8_d, w2l8_d,
          bq_d, bk_d, bvw_d, b1e_d, b2_d, id_d, idb_d, sc_d, outT_d,
          stage_limit, zero_bias):
    mm = nc.tensor.matmul

    def pool(name, bufs, space="SBUF", side=None):
        cm = tc.tile_pool(name=name, bufs=bufs, space=space, side=side)
        return cm, cm.__enter__()

    def close(*cms):
        for cm in cms:
            cm.__exit__(None, None, None)

    # ---------- persistent pools (right stack) ----------
    cpool_cm, cpool = pool("const", 1, side="right")
    w1a_cm, w1ap = pool("w1a", ET // 2, side="right")

    ones128 = cpool.tile([128, 128], dtb)
    nc.vector.memset(ones128[:], 1.0)
    eps128 = cpool.tile([128, 1], dtf)
    nc.vector.memset(eps128[:], 1e-5)
    ident = cpool.tile([128, 128], dtf)
    identb = cpool.tile([128, 128], dtb)
    bq_s = cpool.tile([128, HP], dtf)
    bk_s = cpool.tile([128, HP], dtf)
    bvw_s = cpool.tile([128, D], dtb)
    b1e_s = cpool.tile([128, FT], dtf)
    b2_s = cpool.tile([128, ET], dtf)
    sc_s = cpool.tile([128, 2 + 2 * ET + FT // 4], dtf)
    # (const DMAs are emitted after the startup-critical x/wq loads)

    # PE p-state warmup: ~3us of dummy matmuls during the initial DMA wait
    # so the first real chains run at full clock.
    warm_cm, warmp = pool("warm", 1, space="PSUM")
    wps = warmp.tile([128, 128], dtf)
    for i in range(18):
        mm(wps[:], ones128[:], ones128[:], start=(i == 0), stop=(i == 17))
    close(warm_cm)

    # ---------- left stack: pools living into the attention phase ----------
    # (wo8/oall sit at the bottom so they can outlive hT..kt: LIFO closes)
    wo8_cm, wo8p = pool("wo8", ET // 2)
    oall_cm, oallp = pool("oall", HP // 2)
    hT_cm, hTp = pool("hT", ET)
    v_cm, vp = pool("v", NT)
    qt_cm, qtp = pool("qt", HP)
    wk_cm, wkp = pool("wk", ET)
    kt_cm, ktp = pool("kt", 3)

    # LN-phase pools (wq last/topmost: it frees right after chunk 0's Q)
    wv8_cm, wv8p = pool("wv8", ET // 2)
    h8_cm, h8p = pool("h8", ET)
    xb_cm, xbp = pool("xb", 14)
    sq_cm, sqp = pool("sq", ET + 4)
    ab_cm, abp = pool("ab", 1)
    sps_cm, spsp = pool("sps", 3, space="PSUM")
    qps_cm, qpsp = pool("qps", 2, space="PSUM")
    vps_cm, vpsp = pool("vps", 2, space="PSUM")
    kps0_cm, kps0p = pool("kps0", 1, space="PSUM")
    wq_cm, wqp = pool("wq", ET)

    # DMA order = emission order (single queue): x chunk 0 first, then wq/wv8
    # (needed early), then the rest of x, then wk, wo8, w1 (prefetched
    # through the attention phase).
    def load_xchunk(c):
        out = []
        for e in range(ET):
            t = xbp.tile([128, 512], dtb, tag="xb", name=f"xb{c}_{e}")
            nc.sync.dma_start(out=t[:], in_=xbT_d[ts(e, 128), ts(c, 512)])
            out.append(t)
        return out

    xbc = {0: load_xchunk(0)}
    wq_sb, wv8_sb = [], []
    for e in range(ET):
        tq = wqp.tile([128, D], dtb, tag="wq")
        nc.sync.dma_start(out=tq[:], in_=wq_d[ts(e, 128), :])
        wq_sb.append(tq)
    for i in range(ET // 2):
        tv = wv8p.tile([128, 2, D], dt8, tag="wv8")
        nc.sync.dma_start(out=tv[:], in_=wv8_d[:, ts(i, 2 * D)])
        wv8_sb.append(tv)
    for t_, d_ in ((bq_s, bq_d), (bk_s, bk_d), (bvw_s, bvw_d),
                   (ident, id_d), (identb, idb_d), (b1e_s, b1e_d),
                   (b2_s, b2_d), (sc_s, sc_d)):
        nc.sync.dma_start(out=t_[:], in_=d_[:, :])
    for c in range(1, NCH):
        xbc[c] = load_xchunk(c)
    wk_sb = []
    for e in range(ET):
        tk = wkp.tile([128, 256], dtb, tag="wk")
        nc.sync.dma_start(out=tk[:], in_=wk_d[ts(e, 128), 0:256])
        wk_sb.append(tk)
    wo8_sb = []
    for i in range(ET // 2):
        tw = wo8p.tile([128, 2, D], dt8, tag="wo8")
        nc.sync.dma_start(out=tw[:], in_=wo8_d[:, ts(i, 2 * D)])
        wo8_sb.append(tw)
    W1C = FF - FF // 4
    w1_sb = []
    for e in range(ET // 2):
        tw1 = w1ap.tile([128, W1C], dtb, tag="w1a")
        nc.sync.dma_start(out=tw1[:], in_=w1_d[ts(e, 128), :])
        w1_sb.append(tw1)

    hT = [hTp.tile([128, N], dtb, tag="hT", name=f"hT{e}") for e in range(ET)]
    qt = [qtp.tile([128, T], dtb, tag="qt", name=f"qt{p}") for p in range(HP)]
    v_sb = [vp.tile([128, H * (DH + 1)], dtb, tag="v", name=f"v{j}")
            for j in range(NT)]
    kt_tiles = {}

    def alloc_kt(p):
        kt_tiles[p] = ktp.tile([128, N], dtb, tag="kt", name=f"kt{p}")

    alloc_kt(0)
    alloc_kt(1)

    # ============ stage A+B: per-chunk LN1 -> Q(c0) / V(c) / K0(c) ========
    def ln_normalize(pl, x_slices, S_ps, SQr, out_slices, cw):
        """A = rsqrt(var+eps), B = mean*A; out = x*A - B (all [128, cw])."""
        mean = pl.tile([128, cw], dtf, tag="ab_mean")
        var = pl.tile([128, cw], dtf, tag="ab_var")
        m2 = pl.tile([128, cw], dtf, tag="ab_m2")
        Ar = pl.tile([128, cw], dtf, tag="ab_A")
        Acb = pl.tile([128, cw], dtb, tag="ab_Acb")
        Bcb = pl.tile([128, cw], dtb, tag="ab_Bcb")
        nc.vector.tensor_scalar_mul(mean[:], S_ps[:], 1.0 / D)
        nc.vector.tensor_mul(m2[:], mean[:], mean[:])
        nc.vector.scalar_tensor_tensor(var[:], SQr[:], 1.0 / D, m2[:],
                                       ALU.mult, ALU.subtract)
        nc.scalar.activation(var[:], var[:], AF.Sqrt, bias=eps128[:])
        nc.vector.reciprocal(Ar[:], var[:])
        nc.vector.tensor_mul(Bcb[:], mean[:], Ar[:])   # B = mean*A
        nc.vector.tensor_copy(Acb[:], Ar[:])
        for xsl, osl in zip(x_slices, out_slices):
            nc.vector.tensor_mul(osl, xsl, Acb[:])
            nc.vector.tensor_sub(osl, osl, Bcb[:])

    def squares(sql, x_slices, cw):
        sq = []
        for xsl in x_slices:
            t = sql.tile([128, cw], dtb, tag="sq")
            nc.vector.tensor_mul(t[:], xsl, xsl)
            sq.append(t)
        return sq

    def pe_sum(psp, tag, slices, cw):
        """sum over tiles via all-ones stationary matmul chain (replicated)."""
        s = psp.tile([128, cw], dtf, tag=tag)
        for i, sl in enumerate(slices):
            mm(s[:], ones128[:], sl, start=(i == 0), stop=(i == len(slices) - 1))
        return s

    def wk_slice(e, p):
        if p < 2:
            return wk_sb[e][:, ts(p, 128)]
        return wkB_sb[e][:, ts(p - 2, 128)]

    def k_chain(kt_t, p, c, ps_pool, ps_tag):
        k_ps = ps_pool.tile([128, 512], dtf, tag=ps_tag)
        for e in range(ET):
            mm(k_ps[:], wk_slice(e, p), hT[e][:, ts(c, 512)],
               start=(e == 0), stop=(e == ET - 1))
        if p < 2:
            # LN phase: ACT is congested with h8/V copies and kt0's last
            # bias gates the attention start; DVE has slack here
            nc.vector.tensor_scalar_add(kt_t[:, ts(c, 512)], k_ps[:],
                                        bk_s[:, p:p + 1])
        else:
            nc.scalar.activation(kt_t[:, ts(c, 512)], k_ps[:], AF.Identity,
                                 bias=bk_s[:, p:p + 1])

    def emit_qchain(p):
        q_ps = qpsp.tile([128, T], dtf, tag="qps")
        for e in range(ET):
            mm(q_ps[:], wq_sb[e][:, ts(p, 128)], hT[e][:, 0:T],
               start=(e == 0), stop=(e == ET - 1))
        nc.scalar.activation(qt[p][:], q_ps[:], AF.Identity,
                             bias=bq_s[:, p:p + 1])

    def emit_work(c, h8c):
        """matmul work for chunk c (emitted once chunk c's h is ready):
        Q pairs {2c, 2c+1}, K pairs 0/1, V (fp8 DoubleRow)."""
        emit_qchain(2 * c)
        k_chain(kt_tiles[0], 0, c, kps0p, "kps0")
        emit_qchain(2 * c + 1)
        k_chain(kt_tiles[1], 1, c, kps0p, "kps0")
        for j in range(4 * c, 4 * c + 4):
            vt = v_sb[j]
            v3 = vt[:].rearrange("p (h c) -> p h c", c=DH + 1)
            nc.vector.memset(v3[:, :, DH:DH + 1], 1.0)
            for c2 in range(2):
                v_ps = vpsp.tile([128, 512], dtf, tag="vps")
                for i in range(ET // 2):
                    mm(v_ps[:], h8c[i][:, :, ts(j - 4 * c, 128)],
                       wv8_sb[i][:, :, ts(c2, 512)],
                       start=(i == 0), stop=(i == ET // 2 - 1),
                       perf_mode=DR)
                if zero_bias:
                    # bv == 0: plain descaled copy on the (idle) ACT engine
                    nc.scalar.activation(
                        v3[:, c2 * 8:(c2 + 1) * 8, 0:DH],
                        v_ps[:].rearrange("p (h c) -> p h c", c=DH),
                        AF.Copy, scale=sc_s[:, 0:1])
                else:
                    bsl = bvw_s[:, ts(c2, 512)].rearrange(
                        "p (h c) -> p h c", c=DH)
                    nc.vector.scalar_tensor_tensor(
                        v3[:, c2 * 8:(c2 + 1) * 8, 0:DH],
                        v_ps[:].rearrange("p (h c) -> p h c", c=DH),
                        sc_s[:, 0:1], bsl, ALU.mult, ALU.add)

    # software-pipelined chunk loop: stats of chunk c+1 go into the PE queue
    # BEFORE the matmul work of chunk c, so the (in-order) PE never sits
    # behind a wait for chunk c's normalize.
    prev = None
    for c in range(NCH):
        csl = ts(c, 512)
        xc = [xbc[c][e][:, :] for e in range(ET)]
        S_ps = pe_sum(spsp, "S", xc, 512)
        sq = squares(sqp, xc, 512)
        SQr = pe_sum(spsp, "S", [t[:] for t in sq], 512)
        if prev is not None:
            emit_work(*prev)
        ln_normalize(abp, xc, S_ps, SQr,
                     [hT[e][:, csl] for e in range(ET)], 512)
        # fp8 copy of h (x H8S) for the DoubleRow V projection, on ACT
        h8c = [h8p.tile([128, 2, 512], dt8, tag="h8", name=f"h8_{c}_{i}")
               for i in range(ET // 2)]
        for i in range(ET // 2):
            for jb in range(2):
                nc.scalar.activation(h8c[i][:, jb, :],
                                     hT[2 * i + jb][:, csl],
                                     AF.Copy, scale=H8S)
        prev = (c, h8c)
    emit_work(*prev)
    close(wq_cm)

    close(kps0_cm, vps_cm, qps_cm, sps_cm, ab_cm, sq_cm,
          xb_cm, h8_cm, wv8_cm)
    if stage_limit == "A":
        close(kt_cm, wk_cm, qt_cm, v_cm, hT_cm, oall_cm, wo8_cm,
              w1a_cm, cpool_cm)
        return

    # wk columns for pairs 2-7 (left stack, on top of kt: closes first).
    wkB_cm, wkBp = pool("wkB", ET)
    wkB_sb = []
    for e in range(ET):
        tk = wkBp.tile([128, 768], dtb, tag="wkB")
        nc.sync.dma_start(out=tk[:], in_=wk_d[ts(e, 128), 256:D])
        wkB_sb.append(tk)

    # second half of w1 + residual prefetch (right stack, closed at the very
    # end). DMAs queue behind w1a and land during the attention phase.
    w1b_cm, w1bp = pool("w1b", ET // 2, side="right")
    for e in range(ET // 2, ET):
        tw1 = w1bp.tile([128, W1C], dtb, tag="w1b")
        nc.sync.dma_start(out=tw1[:], in_=w1_d[ts(e, 128), :])
        w1_sb.append(tw1)
    w1h_cm, w1hp = pool("w1h", ET // 2, side="right")
    w1l_cm, w1lp = pool("w1l", ET // 2, side="right")
    w1h_sb, w1l_sb = [], []
    for i in range(ET // 2):
        th = w1hp.tile([128, 2, FF // 4], dt8, tag="w1h")
        nc.sync.dma_start(out=th[:], in_=w1h8_d[:, ts(i, FF // 2)])
        w1h_sb.append(th)
        tl = w1lp.tile([128, 2, FF // 4], dt8, tag="w1l")
        nc.sync.dma_start(out=tl[:], in_=w1l8_d[:, ts(i, FF // 2)])
        w1l_sb.append(tl)
    xo_cm, xop = pool("xo", ET, side="right")
    xo_sb = []
    for e in range(ET):
        tx = xop.tile([128, T], dtb, tag="xo")
        nc.sync.dma_start(out=tx[:], in_=xoT_d[ts(e, 128), :])
        xo_sb.append(tx)

    # ============ stage C: attention, software-pipelined ============
    # Per pair p's j-loop: dots/exp lead, AV lags LAG slots (so pair p-1's
    # oT drains before AV(p,0) needs its PSUM slot), K chains for pair p+2
    # fill PE gaps, and pair p-1's transposes ride the first 8 slots.
    LAG = 6
    pt_cm, ptp = pool("pt", LAG + 3)
    onr_cm, onrp = pool("onr", 12)
    rec_cm, recp = pool("rec", 4)
    dps_cm, dpsp = pool("dps", 2, space="PSUM")
    ops_cm, opsp = pool("ops", 1, space="PSUM")
    kps_cm, kpsp = pool("kps", 1, space="PSUM")
    trp_cm, trpp = pool("trp", 1, space="PSUM")

    oall8 = [oallp.tile([128, 2, T], dt8, tag="oall", name=f"oall{i}")
             for i in range(HP // 2)]

    def emit_av(oT, p, j, pt):
        for h2 in range(2):
            voff = (2 * p + h2) * (DH + 1)
            # one accumulation group per 2KB zero region (bank): start
            # zeroes the whole bank, so the 4 qc-chains share one group
            for qc in range(4):
                mm(oT[:, h2 * 512 + qc * 65: h2 * 512 + qc * 65 + 65],
                   pt[:, h2 * T + qc * 128: h2 * T + (qc + 1) * 128],
                   v_sb[j][:, voff: voff + DH + 1],
                   start=(j == 0 and qc == 0),
                   stop=(j == NT - 1 and qc == 3))

    def emit_norm(oT, p):
        """batched reciprocal + scale for the 8 (head, qchunk) outputs of
        pair p; returns normalized bf16 [128, DH] tiles (x s_oall), which
        get transposed during the next pair."""
        den = recp.tile([128, 8], dtf, tag="den")
        for h2 in range(2):
            dsl = oT[:, h2 * 512: h2 * 512 + 260].rearrange(
                "p (q s) -> p q s", s=DH + 1)[:, :, DH:DH + 1]
            nc.vector.tensor_copy(
                den[:, h2 * 4:(h2 + 1) * 4].rearrange("p (q s) -> p q s", s=1),
                dsl)
        rec = recp.tile([128, 8], dtf, tag="rec")
        nc.vector.reciprocal(rec[:], den[:])
        out = []
        for i in range(8):
            h2, qc = divmod(i, 4)
            base = h2 * 512 + qc * 65
            onr = onrp.tile([128, DH], dtb, tag="onr", name=f"onr{p}_{i}")
            nc.vector.tensor_scalar(onr[:], oT[:, base: base + DH],
                                    rec[:, i:i + 1], sc_s[:, 1:2],
                                    ALU.mult, ALU.mult)
            out.append(onr)
        return out

    def emit_transpose(p, i, onr):
        h2, qc = divmod(i, 4)
        tr = trpp.tile([64, 128], dtb, tag="tr")
        nc.tensor.transpose(tr[:], onr[:], identb[:])
        nc.vector.tensor_copy(
            oall8[p // 2][h2 * DH:(h2 + 1) * DH, p % 2, ts(qc, 128)], tr[:])

    prev_norm = None
    for p in range(HP):
        k_items = []
        if p + 2 < HP:
            alloc_kt(p + 2)
            k_items = [(c, e) for c in range(NCH) for e in range(ET)]
        kt_cur = kt_tiles[p]
        k_ps = None
        oT = opsp.tile([128, 1024], dtf, tag="oT")
        ptq = {}
        for j in range(NT):
            dp = dpsp.tile([128, 2 * T], dtf, tag="dp")
            mm(dp[:, 0:T], kt_cur[0:64, ts(j, 128)], qt[p][0:64, :],
               start=True, stop=True)
            mm(dp[:, T:2 * T], kt_cur[64:128, ts(j, 128)], qt[p][64:128, :],
               start=True, stop=True)
            pt = ptp.tile([128, 2 * T], dtb, tag="pt")
            if j % 3 == 2:
                # Schraudolph exp on DVE: int16(x*2^7/ln2 + 127*2^7) bits
                # read back as bf16 ~= exp(x) (+-2% sawtooth; consistent
                # between numerator and denominator, so softmax cancels it)
                nc.vector.tensor_scalar(pt[:].bitcast(mybir.dt.int16), dp[:],
                                        184.6617, 16249.6,
                                        ALU.mult, ALU.add)
            else:
                nc.scalar.activation(pt[:], dp[:], AF.Exp)
            ptq[j] = pt
            if prev_norm is not None and j < 8:
                emit_transpose(p - 1, j, prev_norm[j])
            if j >= LAG:
                emit_av(oT, p, j - LAG, ptq.pop(j - LAG))
            # interleave 2 K-chain matmuls for pair p+2
            for _ in range(2):
                if not k_items:
                    continue
                c, e = k_items.pop(0)
                if e == 0:
                    k_ps = kpsp.tile([128, 512], dtf, tag="kps")
                mm(k_ps[:], wk_slice(e, p + 2),
                   hT[e][:, ts(c, 512)],
                   start=(e == 0), stop=(e == ET - 1))
                if e == ET - 1:
                    nc.vector.tensor_scalar_add(
                        kt_tiles[p + 2][:, ts(c, 512)], k_ps[:],
                        bk_s[:, p + 2:p + 3])
        for j in range(NT - LAG, NT):
            emit_av(oT, p, j, ptq.pop(j))
        prev_norm = emit_norm(oT, p)
    for i in range(8):
        emit_transpose(HP - 1, i, prev_norm[i])

    close(trp_cm, kps_cm, ops_cm, dps_cm, rec_cm, onr_cm, pt_cm)
    close(wkB_cm, kt_cm, wk_cm, qt_cm, v_cm, hT_cm)
    if stage_limit == "C":
        close(oall_cm, wo8_cm, xo_cm, w1l_cm, w1h_cm, w1b_cm, w1a_cm, cpool_cm)
        return

    # ============ stage D: wo proj (fp8 DR) + residual + LN2 ============
    x2_cm, x2p = pool("x2", ET, side="right")
    h2_cm, h2p = pool("h2", ET, side="right")
    x2b_cm, x2bp = pool("x2b", ET)
    sqd_cm, sqdp = pool("sqd", ET)
    abd_cm, abdp = pool("abd", 1)
    prs_cm, prsp = pool("prs", 2, space="PSUM")
    s2s_cm, s2sp = pool("s2s", 1, space="PSUM")
    sq2s_cm, sq2sp = pool("sq2s", 1, space="PSUM")

    x2, x2b = [], []
    S2_ps = s2sp.tile([128, T], dtf, tag="S2")
    for e in range(ET):
        pr_ps = prsp.tile([128, T], dtf, tag="prs")
        for i in range(ET // 2):
            mm(pr_ps[:], wo8_sb[i][:, :, ts(e, 128)], oall8[i][:, :, :],
               start=(i == 0), stop=(i == ET // 2 - 1), perf_mode=DR)
        tx2 = x2p.tile([128, T], dtf, tag="x2")
        nc.vector.scalar_tensor_tensor(tx2[:], pr_ps[:], sc_s[:, 2 + e:3 + e],
                                       xo_sb[e][:], ALU.mult, ALU.add)
        x2.append(tx2)
        tb = x2bp.tile([128, T], dtb, tag="x2b")
        nc.scalar.activation(tb[:], tx2[:], AF.Copy)
        x2b.append(tb)
        mm(S2_ps[:], ones128[:], tb[:], start=(e == 0), stop=(e == ET - 1))

    h2 = [h2p.tile([128, T], dtb, tag="h2", name=f"h2_{e}")
          for e in range(ET)]
    sq2 = squares(sqdp, [t[:, :] for t in x2b], T)
    SQ2r = pe_sum(sq2sp, "SQ2", [t[:] for t in sq2], T)
    ln_normalize(abdp, [t[:, :] for t in x2b], S2_ps, SQ2r,
                 [t[:, :] for t in h2], T)
    close(sq2s_cm, s2s_cm, prs_cm, abd_cm, sqd_cm, x2b_cm)
    close(oall_cm, wo8_cm)
    if stage_limit == "D":
        close(h2_cm, x2_cm, xo_cm, w1l_cm, w1h_cm, w1b_cm, w1a_cm, cpool_cm)
        return

    # ============ stage E: FFN ============
    # FFN1 bf16; FFN2 entirely as fp8 weight-residual DoubleRow: hi and lo
    # chains share one per-column scale (psum descales once per e-tile),
    # gelu writes g directly in unscaled e4m3.
    FP = FT // 2                         # 16 f-pair tiles
    FQ = FT - FT // 4                    # first bf16 f-tile count (24)
    h28_cm, h28p = pool("h28", ET // 2)
    g8_cm, g8p = pool("g8", FP)
    w2h_cm, w2hp = pool("w2h", FP)
    w2l_cm, w2lp = pool("w2l", FP)
    w2h_sb, w2l_sb = [], []
    for i in range(FP):
        th = w2hp.tile([128, 2, D], dt8, tag="w2h")
        nc.sync.dma_start(out=th[:], in_=w2h8_d[:, ts(i, 2 * D)])
        w2h_sb.append(th)
    for i in range(FP):
        tl = w2lp.tile([128, 2, D], dt8, tag="w2l")
        nc.sync.dma_start(out=tl[:], in_=w2l8_d[:, ts(i, 2 * D)])
        w2l_sb.append(tl)
    aps_cm, apsp = pool("aps", 3, space="PSUM")
    g8_sb = [g8p.tile([128, 2, T], dt8, tag="g8", name=f"g8_{i}")
             for i in range(FP)]
    # fp8 copy of h2 for the residual FFN1 tail tiles (ACT, ahead of the
    # gelu queue so it lands while the bf16 chains run)
    h28 = [h28p.tile([128, 2, T], dt8, tag="h28", name=f"h28_{i}")
           for i in range(ET // 2)]
    for i in range(ET // 2):
        for jb in range(2):
            nc.scalar.activation(h28[i][:, jb, :], h2[2 * i + jb][:],
                                 AF.Copy, scale=H8S)
    for f in range(FT):
        a_ps = apsp.tile([128, T], dtf, tag="aps")
        if f < FQ:
            for e in range(ET):
                mm(a_ps[:], w1_sb[e][:, ts(f, 128)], h2[e][:],
                   start=(e == 0), stop=(e == ET - 1))
            nc.scalar.activation(g8_sb[f // 2][:, f % 2, :], a_ps[:],
                                 AF.Gelu, bias=b1e_s[:, f:f + 1])
        else:
            fr = f - FQ
            for i in range(ET // 2):
                mm(a_ps[:], w1h_sb[i][:, :, ts(fr, 128)], h28[i][:, :, :],
                   start=(i == 0), stop=False, perf_mode=DR)
            for i in range(ET // 2):
                mm(a_ps[:], w1l_sb[i][:, :, ts(fr, 128)], h28[i][:, :, :],
                   start=False, stop=(i == ET // 2 - 1), perf_mode=DR)
            nc.scalar.activation(g8_sb[f // 2][:, f % 2, :], a_ps[:],
                                 AF.Gelu, bias=b1e_s[:, f:f + 1],
                                 scale=sc_s[:, 2 + 2 * ET + fr:3 + 2 * ET + fr])
    close(aps_cm)

    ob_cm, obp = pool("ob", 4)
    yps_cm, ypsp = pool("yps", 3, space="PSUM")
    for e in range(ET):
        y_ps = ypsp.tile([128, T], dtf, tag="yps")
        for i in range(FP):
            mm(y_ps[:], w2h_sb[i][:, :, ts(e, 128)], g8_sb[i][:, :, :],
               start=(i == 0), stop=False, perf_mode=DR)
        for i in range(FP):
            mm(y_ps[:], w2l_sb[i][:, :, ts(e, 128)], g8_sb[i][:, :, :],
               start=False, stop=(i == FP - 1), perf_mode=DR)
        ob = obp.tile([128, T], dtf, tag="ob")
        nc.vector.scalar_tensor_tensor(ob[:], y_ps[:],
                                       sc_s[:, 2 + ET + e:3 + ET + e],
                                       x2[e][:], ALU.mult, ALU.add)
        if not zero_bias:
            nc.vector.tensor_scalar_add(ob[:], ob[:], b2_s[:, e:e + 1])
        nc.sync.dma_start(out=outT_d[ts(e, 128), :], in_=ob[:])
    close(yps_cm, ob_cm, w2l_cm, w2h_cm, g8_cm, h28_cm)

    close(h2_cm, x2_cm, xo_cm, w1l_cm, w1h_cm, w1b_cm, w1a_cm, cpool_cm)


_NC_CACHE = {}


def _zero_bias_flag(ln1_b):
    return bool(np.all(np.asarray(ln1_b) == 0.0))


def _get_nc(zero_bias=False):
    key = ("nc", zero_bias)
    if key not in _NC_CACHE:
        _NC_CACHE[key] = build(zero_bias=zero_bias)
    return _NC_CACHE[key]


def _vec_tiles(v, ntiles):
    return np.ascontiguousarray(
        np.asarray(v, np.float32).reshape(ntiles, 128).T)


def _pair_blocks(w8):
    """[K, cols] quantized array -> [128, (K//256)*2*cols] pair-block layout."""
    blocks = []
    for i in range(w8.shape[0] // 256):
        for j in range(2):
            blocks.append(w8[(2 * i + j) * 128:(2 * i + j + 1) * 128, :])
    return np.ascontiguousarray(np.concatenate(blocks, axis=1))


def _fp8_pairs(w, colscale):
    """[D, D] fp32 -> [128, (D//256)*2*D] e4m3 pair-block layout."""
    return _pair_blocks((w * colscale).astype(ml_dtypes.float8_e4m3))


def prepare_in_maps(x, wq, wk, wv, wo, w1, b1, w2, b2,
                    ln1_g, ln1_b, ln2_g, ln2_b):
    bf = ml_dtypes.bfloat16
    f32 = np.float32
    x = np.asarray(x, f32)
    wq = np.asarray(wq, f32); wk = np.asarray(wk, f32)
    wv = np.asarray(wv, f32); w1 = np.asarray(w1, f32)
    wo = np.asarray(wo, f32)
    g1 = np.asarray(ln1_g, f32)[:, None]
    b1v = np.asarray(ln1_b, f32)
    g2 = np.asarray(ln2_g, f32)[:, None]
    b2v = np.asarray(ln2_b, f32)
    bq = (b1v @ wq).astype(f32)          # [D] per-output-col constants
    bk = (b1v @ wk).astype(f32)
    bv = (b1v @ wv).astype(f32)
    b1eff = (np.asarray(b1, f32) + b2v @ w1).astype(f32)

    wv_g = wv * g1
    s_wv = 120.0 / max(1e-30, np.abs(wv_g).max())
    # bound on |attn out| <= max |v| row; scale so fp8 oall stays in range
    vbound = 6.0 * np.linalg.norm(wv_g, axis=0).max() + np.abs(bv).max()
    s_oall = 120.0 / max(1e-30, vbound)
    s_wocol = 120.0 / np.maximum(np.abs(wo).max(axis=0), 1e-30)
    # FFN2 split: rows 0..FF/2 as fp8 hi+lo residual, rest bf16 pre-scaled
    # by the shared per-column scale s2col (g stays in unscaled e4m3)
    w2f = np.asarray(w2, f32)
    s2col = 120.0 / np.maximum(np.abs(w2f).max(axis=0), 1e-30)
    f8t = ml_dtypes.float8_e4m3
    w2s = w2f * s2col[None, :]
    w2hi = np.asarray(w2s, f8t)
    w2lo = np.asarray(w2s - w2hi.astype(f32), f8t)
    # FFN1 residual tail (f-cols 3072..4095): hi+lo fp8 with per-col scale
    w1_g = np.asarray(w1, f32) * g2
    s1col = 120.0 / np.maximum(np.abs(w1_g[:, 3 * FF // 4:]).max(axis=0),
                               1e-30)
    w1s = w1_g[:, 3 * FF // 4:] * s1col[None, :]
    w1hi = np.asarray(w1s, f8t)
    w1lo = np.asarray(w1s - w1hi.astype(f32), f8t)
    sc = np.zeros((128, 2 + 2 * ET + FT // 4), f32)
    for fr in range(FT // 4):
        sc[:, 2 + 2 * ET + fr] = 1.0 / (H8S * s1col[fr * 128:(fr + 1) * 128])
    sc[:, 0] = 1.0 / (H8S * s_wv)
    sc[:, 1] = s_oall
    for e in range(ET):
        sc[:, 2 + e] = 1.0 / (s_oall * s_wocol[e * 128:(e + 1) * 128])
        sc[:, 2 + ET + e] = 1.0 / s2col[e * 128:(e + 1) * 128]

    shared = {
        "wq": np.ascontiguousarray((wq * g1).astype(bf)),
        "wk": np.ascontiguousarray((wk * g1).astype(bf)),
        "wv8": _fp8_pairs(wv_g, s_wv),
        "wo8": _fp8_pairs(wo, s_wocol[None, :]),
        "w1": np.ascontiguousarray(w1_g[:, :3 * FF // 4].astype(bf)),
        "w1h8": _pair_blocks(w1hi),
        "w1l8": _pair_blocks(w1lo),
        "w2h8": _pair_blocks(w2hi),
        "w2l8": _pair_blocks(w2lo),
        "bq": np.ascontiguousarray(bq.reshape(HP, 128).T),
        "bk": np.ascontiguousarray(bk.reshape(HP, 128).T),
        "bvw": np.ascontiguousarray(np.tile(bv.astype(bf), (128, 1))),
        "b1e": _vec_tiles(b1eff, FT),
        "b2": _vec_tiles(b2, ET),
        "ident": np.ascontiguousarray(np.eye(128, dtype=f32)),
        "identb": np.ascontiguousarray(np.eye(128, dtype=bf)),
        "sc": np.ascontiguousarray(sc),
    }
    in_maps = []
    for c in range(NCORES):
        b, s = divmod(c, CPB)
        rot = np.concatenate([x[b, s * T:], x[b, :s * T]], axis=0)  # own first
        m = dict(shared)
        m["xbT"] = np.ascontiguousarray(rot.T.astype(bf))
        m["xoT"] = np.ascontiguousarray(x[b, s * T:(s + 1) * T].T.astype(bf))
        in_maps.append(m)
    return in_maps


def assemble_output(results):
    out = np.empty((B, N, D), np.float32)
    for c in range(NCORES):
        b, s = divmod(c, CPB)
        out[b, s * T:(s + 1) * T, :] = results[c]["outT"].T
    return out


def kernel(x, wq, wk, wv, wo, w1, b1, w2, b2, ln1_g, ln1_b, ln2_g, ln2_b):
    from concourse.bass_utils import run_bass_kernel_spmd

    nc = _get_nc(_zero_bias_flag(ln1_b))
    in_maps = prepare_in_maps(x, wq, wk, wv, wo, w1, b1, w2, b2,
                              ln1_g, ln1_b, ln2_g, ln2_b)
    res = run_bass_kernel_spmd(nc, in_maps, core_ids=list(range(NCORES)))
    return assemble_output(res.results)



# revision 105
# speedup vs baseline: 1.0259x; 1.0045x over previous
"""Trainium2 Bass kernel for a pre-norm transformer encoder layer.

Problem: x[2,2048,1024]; LN1 -> QKV (16 heads x 64) -> softmax(QK^T) V
-> wo -> +res -> LN2 -> GELU(h@w1+b1)@w2+b2 -> +res.

Sharding: token-parallel over B*N = 4096 tokens; each of the 8 cores owns
512 tokens (cores 0-3: batch 0, cores 4-7: batch 1). Each core recomputes
K/V for its whole batch (no collectives). All activations are kept in
transposed layout [feature, token] so every matmul contracts over the
partition dim. Host pre-rotates each core's batch so its own 512 tokens
are always columns 0:512 -> one NEFF shared by all 8 cores.

v3 restructure (fp8-DoubleRow on the error-tolerant matmuls, multi-engine
softmax, latency-pipelined phases):
 - LN gammas are folded into wq/wk/wv/w1 rows on the host; LN betas become
   per-output-feature constants (b@W) applied for free as per-partition
   bias APs in the PSUM->SBUF copies. Device LN is (x-mu)*rstd = x*A - B.
 - LN statistics (sum, sum-of-squares) both via all-ones [128,128]
   stationary PE matmul chains (replicated output); the chunk loop is
   software-pipelined so chunk c+1's stats sit in the (in-order) PE queue
   before chunk c's Q/K/V work and the PE never idles on a normalize.
 - V projection and the wo projection run as fp8(e4m3) DoubleRow matmuls
   (0.5 cycles/col, contraction 256/matmul): host pre-packs wv/wo in
   [128,2,cols] pair-block layout with per-column scales; hT gets an fp8
   copy (x16) on the ACT engine; Q/K/dots/AV/FFN stay bf16 (fp8 there
   fails the 2e-2 gate -- measured per-matmul on the reference).
   Runtime descales travel in a small "sc" constant tensor as
   per-partition scalar APs fused into scalar_tensor_tensor ops.
 - Attention AV uses pt (exp dots, keys on partitions) as the *stationary*
   operand so the output is [128 queries, 65]; softmax denominators are
   gathered strided, one batched reciprocal per pair, normalize fused
   (x rec x s_oall) into a single two-scalar tensor_scalar writing bf16;
   bf16 PE transposes restore [dh, token]; transposed tiles land directly
   in the fp8 pair-block oall operand of the wo DoubleRow matmul.
 - softmax exp is split across engines: 2/3 on ACT (AF.Exp), 1/3 on DVE
   via a Schraudolph bit-trick exp -- int16(x*2^7/ln2 + 127*2^7) written
   through a bitcast AP and read back as bf16 (~+-2% sawtooth, identical
   in numerator and denominator so softmax normalizes it away).
 - The attention j-loop is software-pipelined as in v2 (dots/exp lead,
   AV lags LAG slots, K for pair p+2 interleaved 2 matmuls/slot).
 - w1 is prefetched through the LN/attention phases (split across both
   SBUF stacks), wk is split so only pairs 0/1 occupy SBUF during LN,
   and 18 warmup matmuls during the initial DMA raise the PE p-state
   before the first real chain.
 - FFN2 (all 32 f-tiles) and the last quarter of FFN1 run as fp8
   weight-residual DoubleRow: weights = hi+lo e4m3 pair-blocks sharing one
   per-column scale (~0.1% weight error, 2x fewer PE cycles than bf16).
   gelu writes g directly in unscaled e4m3; h2 gets an fp8 copy (x16) on
   ACT ahead of the gelu queue; hi+lo chains accumulate into one psum and
   descale once (FFN2: in the output stt; FFN1: in the gelu scale AP).
   Residual fp8 adds activation-quantization error only.
 - FFN runs f-outer then e-outer so output tiles drain early.

Matmuls accumulate in fp32 PSUM. Cost-model time: 324.0us (baseline 420.2);
device rel err 1.7214e-2 (gate 2e-2).
"""
import sys
sys.path.insert(0, "/opt/trn_rl_repo")

import numpy as np
import ml_dtypes

import concourse.bass as bass
import concourse.bass_isa as bass_isa
import concourse.tile as tile
from concourse import bacc, mybir

B, N, D = 2, 2048, 1024
H, DH = 16, 64
FF = 4096
NCORES = 8
T = N * B // NCORES          # 512 tokens per core
CPB = NCORES // B            # 4 cores per batch
ET = D // 128                # 8 embed tiles
FT = FF // 128               # 32 ffn tiles
NT = N // 128                # 16 key tiles per batch
NCH = N // 512               # 4 512-chunks per batch
HP = H // 2                  # 8 head pairs

dtb = mybir.dt.bfloat16
dtf = mybir.dt.float32
dt8 = mybir.dt.float8e4
AF = mybir.ActivationFunctionType
RED = bass_isa.ReduceOp
DR = mybir.MatmulPerfMode.DoubleRow
ALU = mybir.AluOpType
ts = bass.ts
H8S = 16.0                   # static scale for the fp8 copy of hT (h ~ N(0,1))


def build(stage_limit="E", zero_bias=False):
    nc = bacc.Bacc("TRN2", target_bir_lowering=False, debug=False)

    xbT_d = nc.dram_tensor("xbT", [D, N], dtb, kind="ExternalInput").ap()
    xoT_d = nc.dram_tensor("xoT", [D, T], dtb, kind="ExternalInput").ap()
    wq_d = nc.dram_tensor("wq", [D, D], dtb, kind="ExternalInput").ap()
    wk_d = nc.dram_tensor("wk", [D, D], dtb, kind="ExternalInput").ap()
    wv8_d = nc.dram_tensor("wv8", [128, ET * D], dt8, kind="ExternalInput").ap()
    wo8_d = nc.dram_tensor("wo8", [128, ET * D], dt8, kind="ExternalInput").ap()
    w1_d = nc.dram_tensor("w1", [D, FF - FF // 4], dtb,
                          kind="ExternalInput").ap()
    w1h8_d = nc.dram_tensor("w1h8", [128, (ET // 2) * 2 * (FF // 4)], dt8,
                            kind="ExternalInput").ap()
    w1l8_d = nc.dram_tensor("w1l8", [128, (ET // 2) * 2 * (FF // 4)], dt8,
                            kind="ExternalInput").ap()
    # FFN2 as fp8 weight-residual (hi+lo DoubleRow chains, ~0.1% weight
    # error, one shared per-column scale, activation g in unscaled e4m3)
    w2h8_d = nc.dram_tensor("w2h8", [128, (FT // 2) * 2 * D], dt8,
                            kind="ExternalInput").ap()
    w2l8_d = nc.dram_tensor("w2l8", [128, (FT // 2) * 2 * D], dt8,
                            kind="ExternalInput").ap()
    bq_d = nc.dram_tensor("bq", [128, HP], dtf, kind="ExternalInput").ap()
    bk_d = nc.dram_tensor("bk", [128, HP], dtf, kind="ExternalInput").ap()
    bvw_d = nc.dram_tensor("bvw", [128, D], dtb, kind="ExternalInput").ap()
    b1e_d = nc.dram_tensor("b1e", [128, FT], dtf, kind="ExternalInput").ap()
    b2_d = nc.dram_tensor("b2", [128, ET], dtf, kind="ExternalInput").ap()
    id_d = nc.dram_tensor("ident", [128, 128], dtf, kind="ExternalInput").ap()
    idb_d = nc.dram_tensor("identb", [128, 128], dtb, kind="ExternalInput").ap()
    # sc: col0 = v descale 1/(H8S*s_wv); col1 = s_oall; cols 2..9 = wo
    # per-feature descale 1/(s_oall*s_wocol) per e-tile
    sc_d = nc.dram_tensor("sc", [128, 2 + 2 * ET + FT // 4], dtf, kind="ExternalInput").ap()
    outT_d = nc.dram_tensor("outT", [D, T], dtf, kind="ExternalOutput").ap()

    with tile.TileContext(nc) as tc:
        _body(nc, tc, xbT_d, xoT_d, wq_d, wk_d, wv8_d, wo8_d, w1_d,
              w1h8_d, w1l8_d, w2h8_d, w2l8_d,
              bq_d, bk_d, bvw_d, b1e_d, b2_d, id_d, idb_d, sc_d, outT_d,
              stage_limit, zero_bias)
    nc.finalize()
    return nc


def _body(nc, tc, xbT_d, xoT_d, wq_d, wk_d, wv8_d, wo8_d, w1_d,
          w1h8_d, w1l8_d, w2h8_d, w2l8_d,
          bq_d, bk_d, bvw_d, b1e_d, b2_d, id_d, idb_d, sc_d, outT_d,
          stage_limit, zero_bias):
    mm = nc.tensor.matmul

    def pool(name, bufs, space="SBUF", side=None):
        cm = tc.tile_pool(name=name, bufs=bufs, space=space, side=side)
        return cm, cm.__enter__()

    def close(*cms):
        for cm in cms:
            cm.__exit__(None, None, None)

    # ---------- persistent pools (right stack) ----------
    cpool_cm, cpool = pool("const", 1, side="right")
    w1a_cm, w1ap = pool("w1a", ET // 2, side="right")

    ones128 = cpool.tile([128, 128], dtb)
    nc.vector.memset(ones128[:], 1.0)
    eps128 = cpool.tile([128, 1], dtf)
    nc.vector.memset(eps128[:], 1e-5)
    ident = cpool.tile([128, 128], dtf)
    identb = cpool.tile([128, 128], dtb)
    bq_s = cpool.tile([128, HP], dtf)
    bk_s = cpool.tile([128, HP], dtf)
    bvw_s = cpool.tile([128, D], dtb)
    b1e_s = cpool.tile([128, FT], dtf)
    b2_s = cpool.tile([128, ET], dtf)
    sc_s = cpool.tile([128, 2 + 2 * ET + FT // 4], dtf)
    # (const DMAs are emitted after the startup-critical x/wq loads)

    # PE p-state warmup: ~3us of dummy matmuls during the initial DMA wait
    # so the first real chains run at full clock.
    warm_cm, warmp = pool("warm", 1, space="PSUM")
    wps = warmp.tile([128, 128], dtf)
    for i in range(18):
        mm(wps[:], ones128[:], ones128[:], start=(i == 0), stop=(i == 17))
    close(warm_cm)

    # ---------- left stack: pools living into the attention phase ----------
    # (wo8/oall sit at the bottom so they can outlive hT..kt: LIFO closes)
    wo8_cm, wo8p = pool("wo8", ET // 2)
    oall_cm, oallp = pool("oall", HP // 2)
    hT_cm, hTp = pool("hT", ET)
    v_cm, vp = pool("v", NT)
    qt_cm, qtp = pool("qt", HP)
    wk_cm, wkp = pool("wk", ET)
    kt_cm, ktp = pool("kt", 3)

    # LN-phase pools (wq last/topmost: it frees right after chunk 0's Q)
    wv8_cm, wv8p = pool("wv8", ET // 2)
    h8_cm, h8p = pool("h8", ET)
    xb_cm, xbp = pool("xb", 2)
    sq_cm, sqp = pool("sq", ET + 4)
    ab_cm, abp = pool("ab", 1)
    sps_cm, spsp = pool("sps", 3, space="PSUM")
    qps_cm, qpsp = pool("qps", 2, space="PSUM")
    vps_cm, vpsp = pool("vps", 2, space="PSUM")
    kps0_cm, kps0p = pool("kps0", 1, space="PSUM")
    wq_cm, wqp = pool("wq", ET)

    # DMA order = emission order (single queue): x chunk 0 first, then wq/wv8
    # (needed early), then the rest of x, then wk, wo8, w1 (prefetched
    # through the attention phase).
    def load_xchunk(c):
        """one partition-wrapped DMA per 512-token chunk (8 e-tiles)"""
        t = xbp.tile([128, ET, 512], dtb, tag="xb", name=f"xb{c}")
        nc.sync.dma_start(
            out=t[:],
            in_=xbT_d[:, ts(c, 512)].rearrange("(i p) c -> p i c", p=128))
        return [t[:, e, :] for e in range(ET)]

    xbc = {0: load_xchunk(0)}
    wq_sb, wv8_sb = [], []
    for e in range(ET):
        tq = wqp.tile([128, D], dtb, tag="wq")
        nc.sync.dma_start(out=tq[:], in_=wq_d[ts(e, 128), :])
        wq_sb.append(tq)
    for i in range(ET // 2):
        tv = wv8p.tile([128, 2, D], dt8, tag="wv8")
        nc.sync.dma_start(out=tv[:], in_=wv8_d[:, ts(i, 2 * D)])
        wv8_sb.append(tv)
    for t_, d_ in ((bq_s, bq_d), (bk_s, bk_d), (bvw_s, bvw_d),
                   (ident, id_d), (identb, idb_d), (b1e_s, b1e_d),
                   (b2_s, b2_d), (sc_s, sc_d)):
        nc.sync.dma_start(out=t_[:], in_=d_[:, :])
    for c in range(1, NCH):
        xbc[c] = load_xchunk(c)
    wk_sb = []
    for e in range(ET):
        tk = wkp.tile([128, 256], dtb, tag="wk")
        nc.sync.dma_start(out=tk[:], in_=wk_d[ts(e, 128), 0:256])
        wk_sb.append(tk)
    wo8_sb = []
    for i in range(ET // 2):
        tw = wo8p.tile([128, 2, D], dt8, tag="wo8")
        nc.sync.dma_start(out=tw[:], in_=wo8_d[:, ts(i, 2 * D)])
        wo8_sb.append(tw)
    W1C = FF - FF // 4
    w1_sb = []
    for e in range(ET // 2):
        tw1 = w1ap.tile([128, W1C], dtb, tag="w1a")
        nc.sync.dma_start(out=tw1[:], in_=w1_d[ts(e, 128), :])
        w1_sb.append(tw1)

    hT = [hTp.tile([128, N], dtb, tag="hT", name=f"hT{e}") for e in range(ET)]
    qt = [qtp.tile([128, T], dtb, tag="qt", name=f"qt{p}") for p in range(HP)]
    v_sb = [vp.tile([128, H * (DH + 1)], dtb, tag="v", name=f"v{j}")
            for j in range(NT)]
    kt_tiles = {}

    def alloc_kt(p):
        kt_tiles[p] = ktp.tile([128, N], dtb, tag="kt", name=f"kt{p}")

    alloc_kt(0)
    alloc_kt(1)

    # ============ stage A+B: per-chunk LN1 -> Q(c0) / V(c) / K0(c) ========
    def ln_normalize(pl, x_slices, S_ps, SQr, out_slices, cw):
        """A = rsqrt(var+eps), B = mean*A; out = x*A - B (all [128, cw])."""
        mean = pl.tile([128, cw], dtf, tag="ab_mean")
        var = pl.tile([128, cw], dtf, tag="ab_var")
        m2 = pl.tile([128, cw], dtf, tag="ab_m2")
        Ar = pl.tile([128, cw], dtf, tag="ab_A")
        Acb = pl.tile([128, cw], dtb, tag="ab_Acb")
        Bcb = pl.tile([128, cw], dtb, tag="ab_Bcb")
        nc.vector.tensor_scalar_mul(mean[:], S_ps[:], 1.0 / D)
        nc.vector.tensor_mul(m2[:], mean[:], mean[:])
        nc.vector.scalar_tensor_tensor(var[:], SQr[:], 1.0 / D, m2[:],
                                       ALU.mult, ALU.subtract)
        nc.scalar.activation(var[:], var[:], AF.Sqrt, bias=eps128[:])
        nc.vector.reciprocal(Ar[:], var[:])
        nc.vector.tensor_mul(Bcb[:], mean[:], Ar[:])   # B = mean*A
        nc.vector.tensor_copy(Acb[:], Ar[:])
        for xsl, osl in zip(x_slices, out_slices):
            nc.vector.tensor_mul(osl, xsl, Acb[:])
            nc.vector.tensor_sub(osl, osl, Bcb[:])

    def squares(sql, x_slices, cw):
        sq = []
        for xsl in x_slices:
            t = sql.tile([128, cw], dtb, tag="sq")
            nc.vector.tensor_mul(t[:], xsl, xsl)
            sq.append(t)
        return sq

    def pe_sum(psp, tag, slices, cw):
        """sum over tiles via all-ones stationary matmul chain (replicated)."""
        s = psp.tile([128, cw], dtf, tag=tag)
        for i, sl in enumerate(slices):
            mm(s[:], ones128[:], sl, start=(i == 0), stop=(i == len(slices) - 1))
        return s

    def wk_slice(e, p):
        if p < 2:
            return wk_sb[e][:, ts(p, 128)]
        return wkB_sb[e][:, ts(p - 2, 128)]

    def k_chain(kt_t, p, c, ps_pool, ps_tag):
        k_ps = ps_pool.tile([128, 512], dtf, tag=ps_tag)
        for e in range(ET):
            mm(k_ps[:], wk_slice(e, p), hT[e][:, ts(c, 512)],
               start=(e == 0), stop=(e == ET - 1))
        if p < 2:
            # LN phase: ACT is congested with h8/V copies and kt0's last
            # bias gates the attention start; DVE has slack here
            nc.vector.tensor_scalar_add(kt_t[:, ts(c, 512)], k_ps[:],
                                        bk_s[:, p:p + 1])
        else:
            nc.scalar.activation(kt_t[:, ts(c, 512)], k_ps[:], AF.Identity,
                                 bias=bk_s[:, p:p + 1])

    def emit_qchain(p):
        q_ps = qpsp.tile([128, T], dtf, tag="qps")
        for e in range(ET):
            mm(q_ps[:], wq_sb[e][:, ts(p, 128)], hT[e][:, 0:T],
               start=(e == 0), stop=(e == ET - 1))
        nc.scalar.activation(qt[p][:], q_ps[:], AF.Identity,
                             bias=bq_s[:, p:p + 1])

    def emit_work(c, h8c):
        """matmul work for chunk c (emitted once chunk c's h is ready):
        Q pairs {2c, 2c+1}, K pairs 0/1, V (fp8 DoubleRow)."""
        emit_qchain(2 * c)
        k_chain(kt_tiles[0], 0, c, kps0p, "kps0")
        emit_qchain(2 * c + 1)
        k_chain(kt_tiles[1], 1, c, kps0p, "kps0")
        for j in range(4 * c, 4 * c + 4):
            vt = v_sb[j]
            v3 = vt[:].rearrange("p (h c) -> p h c", c=DH + 1)
            nc.vector.memset(v3[:, :, DH:DH + 1], 1.0)
            for c2 in range(2):
                v_ps = vpsp.tile([128, 512], dtf, tag="vps")
                for i in range(ET // 2):
                    mm(v_ps[:], h8c[i][:, :, ts(j - 4 * c, 128)],
                       wv8_sb[i][:, :, ts(c2, 512)],
                       start=(i == 0), stop=(i == ET // 2 - 1),
                       perf_mode=DR)
                if zero_bias:
                    # bv == 0: plain descaled copy on the (idle) ACT engine
                    nc.scalar.activation(
                        v3[:, c2 * 8:(c2 + 1) * 8, 0:DH],
                        v_ps[:].rearrange("p (h c) -> p h c", c=DH),
                        AF.Copy, scale=sc_s[:, 0:1])
                else:
                    bsl = bvw_s[:, ts(c2, 512)].rearrange(
                        "p (h c) -> p h c", c=DH)
                    nc.vector.scalar_tensor_tensor(
                        v3[:, c2 * 8:(c2 + 1) * 8, 0:DH],
                        v_ps[:].rearrange("p (h c) -> p h c", c=DH),
                        sc_s[:, 0:1], bsl, ALU.mult, ALU.add)

    # software-pipelined chunk loop: stats of chunk c+1 go into the PE queue
    # BEFORE the matmul work of chunk c, so the (in-order) PE never sits
    # behind a wait for chunk c's normalize.
    prev = None
    for c in range(NCH):
        csl = ts(c, 512)
        xc = list(xbc[c])
        S_ps = pe_sum(spsp, "S", xc, 512)
        sq = squares(sqp, xc, 512)
        SQr = pe_sum(spsp, "S", [t[:] for t in sq], 512)
        if prev is not None:
            emit_work(*prev)
        ln_normalize(abp, xc, S_ps, SQr,
                     [hT[e][:, csl] for e in range(ET)], 512)
        # fp8 copy of h (x H8S) for the DoubleRow V projection, on ACT
        h8c = [h8p.tile([128, 2, 512], dt8, tag="h8", name=f"h8_{c}_{i}")
               for i in range(ET // 2)]
        for i in range(ET // 2):
            for jb in range(2):
                nc.scalar.activation(h8c[i][:, jb, :],
                                     hT[2 * i + jb][:, csl],
                                     AF.Copy, scale=H8S)
        prev = (c, h8c)
    emit_work(*prev)
    close(wq_cm)

    close(kps0_cm, vps_cm, qps_cm, sps_cm, ab_cm, sq_cm,
          xb_cm, h8_cm, wv8_cm)
    if stage_limit == "A":
        close(kt_cm, wk_cm, qt_cm, v_cm, hT_cm, oall_cm, wo8_cm,
              w1a_cm, cpool_cm)
        return

    # wk columns for pairs 2-7 (left stack, on top of kt: closes first).
    wkB_cm, wkBp = pool("wkB", ET)
    wkB_sb = []
    for e in range(ET):
        tk = wkBp.tile([128, 768], dtb, tag="wkB")
        nc.sync.dma_start(out=tk[:], in_=wk_d[ts(e, 128), 256:D])
        wkB_sb.append(tk)

    # second half of w1 + residual prefetch (right stack, closed at the very
    # end). DMAs queue behind w1a and land during the attention phase.
    w1b_cm, w1bp = pool("w1b", ET // 2, side="right")
    for e in range(ET // 2, ET):
        tw1 = w1bp.tile([128, W1C], dtb, tag="w1b")
        nc.sync.dma_start(out=tw1[:], in_=w1_d[ts(e, 128), :])
        w1_sb.append(tw1)
    w1h_cm, w1hp = pool("w1h", ET // 2, side="right")
    w1l_cm, w1lp = pool("w1l", ET // 2, side="right")
    w1h_sb, w1l_sb = [], []
    for i in range(ET // 2):
        th = w1hp.tile([128, 2, FF // 4], dt8, tag="w1h")
        nc.sync.dma_start(out=th[:], in_=w1h8_d[:, ts(i, FF // 2)])
        w1h_sb.append(th)
        tl = w1lp.tile([128, 2, FF // 4], dt8, tag="w1l")
        nc.sync.dma_start(out=tl[:], in_=w1l8_d[:, ts(i, FF // 2)])
        w1l_sb.append(tl)
    xo_cm, xop = pool("xo", ET, side="right")
    xo_sb = []
    for e in range(ET):
        tx = xop.tile([128, T], dtb, tag="xo")
        nc.sync.dma_start(out=tx[:], in_=xoT_d[ts(e, 128), :])
        xo_sb.append(tx)

    # ============ stage C: attention, software-pipelined ============
    # Per pair p's j-loop: dots/exp lead, AV lags LAG slots (so pair p-1's
    # oT drains before AV(p,0) needs its PSUM slot), K chains for pair p+2
    # fill PE gaps, and pair p-1's transposes ride the first 8 slots.
    LAG = 6
    pt_cm, ptp = pool("pt", LAG + 3)
    onr_cm, onrp = pool("onr", 12)
    rec_cm, recp = pool("rec", 4)
    dps_cm, dpsp = pool("dps", 2, space="PSUM")
    ops_cm, opsp = pool("ops", 1, space="PSUM")
    kps_cm, kpsp = pool("kps", 1, space="PSUM")
    trp_cm, trpp = pool("trp", 1, space="PSUM")

    oall8 = [oallp.tile([128, 2, T], dt8, tag="oall", name=f"oall{i}")
             for i in range(HP // 2)]

    def emit_av(oT, p, j, pt):
        for h2 in range(2):
            voff = (2 * p + h2) * (DH + 1)
            # one accumulation group per 2KB zero region (bank): start
            # zeroes the whole bank, so the 4 qc-chains share one group
            for qc in range(4):
                mm(oT[:, h2 * 512 + qc * 65: h2 * 512 + qc * 65 + 65],
                   pt[:, h2 * T + qc * 128: h2 * T + (qc + 1) * 128],
                   v_sb[j][:, voff: voff + DH + 1],
                   start=(j == 0 and qc == 0),
                   stop=(j == NT - 1 and qc == 3))

    def emit_norm(oT, p):
        """batched reciprocal + scale for the 8 (head, qchunk) outputs of
        pair p; returns normalized bf16 [128, DH] tiles (x s_oall), which
        get transposed during the next pair."""
        den = recp.tile([128, 8], dtf, tag="den")
        for h2 in range(2):
            dsl = oT[:, h2 * 512: h2 * 512 + 260].rearrange(
                "p (q s) -> p q s", s=DH + 1)[:, :, DH:DH + 1]
            nc.vector.tensor_copy(
                den[:, h2 * 4:(h2 + 1) * 4].rearrange("p (q s) -> p q s", s=1),
                dsl)
        rec = recp.tile([128, 8], dtf, tag="rec")
        nc.vector.reciprocal(rec[:], den[:])
        out = []
        for i in range(8):
            h2, qc = divmod(i, 4)
            base = h2 * 512 + qc * 65
            onr = onrp.tile([128, DH], dtb, tag="onr", name=f"onr{p}_{i}")
            nc.vector.tensor_scalar(onr[:], oT[:, base: base + DH],
                                    rec[:, i:i + 1], sc_s[:, 1:2],
                                    ALU.mult, ALU.mult)
            out.append(onr)
        return out

    def emit_transpose(p, i, onr):
        h2, qc = divmod(i, 4)
        tr = trpp.tile([64, 128], dtb, tag="tr")
        nc.tensor.transpose(tr[:], onr[:], identb[:])
        nc.vector.tensor_copy(
            oall8[p // 2][h2 * DH:(h2 + 1) * DH, p % 2, ts(qc, 128)], tr[:])

    prev_norm = None
    for p in range(HP):
        k_items = []
        if p + 2 < HP:
            alloc_kt(p + 2)
            k_items = [(c, e) for c in range(NCH) for e in range(ET)]
        kt_cur = kt_tiles[p]
        k_ps = None
        oT = opsp.tile([128, 1024], dtf, tag="oT")
        ptq = {}
        for j in range(NT):
            dp = dpsp.tile([128, 2 * T], dtf, tag="dp")
            mm(dp[:, 0:T], kt_cur[0:64, ts(j, 128)], qt[p][0:64, :],
               start=True, stop=True)
            mm(dp[:, T:2 * T], kt_cur[64:128, ts(j, 128)], qt[p][64:128, :],
               start=True, stop=True)
            pt = ptp.tile([128, 2 * T], dtb, tag="pt")
            if j % 3 == 2:
                # Schraudolph exp on DVE: int16(x*2^7/ln2 + 127*2^7) bits
                # read back as bf16 ~= exp(x) (+-2% sawtooth; consistent
                # between numerator and denominator, so softmax cancels it)
                nc.vector.tensor_scalar(pt[:].bitcast(mybir.dt.int16), dp[:],
                                        184.6617, 16249.6,
                                        ALU.mult, ALU.add)
            else:
                nc.scalar.activation(pt[:], dp[:], AF.Exp)
            ptq[j] = pt
            if prev_norm is not None and j < 8:
                emit_transpose(p - 1, j, prev_norm[j])
            if j >= LAG:
                emit_av(oT, p, j - LAG, ptq.pop(j - LAG))
            # interleave 2 K-chain matmuls for pair p+2
            for _ in range(2):
                if not k_items:
                    continue
                c, e = k_items.pop(0)
                if e == 0:
                    k_ps = kpsp.tile([128, 512], dtf, tag="kps")
                mm(k_ps[:], wk_slice(e, p + 2),
                   hT[e][:, ts(c, 512)],
                   start=(e == 0), stop=(e == ET - 1))
                if e == ET - 1:
                    nc.vector.tensor_scalar_add(
                        kt_tiles[p + 2][:, ts(c, 512)], k_ps[:],
                        bk_s[:, p + 2:p + 3])
        for j in range(NT - LAG, NT):
            emit_av(oT, p, j, ptq.pop(j))
        prev_norm = emit_norm(oT, p)
    for i in range(8):
        emit_transpose(HP - 1, i, prev_norm[i])

    close(trp_cm, kps_cm, ops_cm, dps_cm, rec_cm, onr_cm, pt_cm)
    close(wkB_cm, kt_cm, wk_cm, qt_cm, v_cm, hT_cm)
    if stage_limit == "C":
        close(oall_cm, wo8_cm, xo_cm, w1l_cm, w1h_cm, w1b_cm, w1a_cm, cpool_cm)
        return

    # ============ stage D: wo proj (fp8 DR) + residual + LN2 ============
    x2_cm, x2p = pool("x2", ET, side="right")
    h2_cm, h2p = pool("h2", ET, side="right")
    x2b_cm, x2bp = pool("x2b", ET)
    sqd_cm, sqdp = pool("sqd", ET)
    abd_cm, abdp = pool("abd", 1)
    prs_cm, prsp = pool("prs", 2, space="PSUM")
    s2s_cm, s2sp = pool("s2s", 1, space="PSUM")
    sq2s_cm, sq2sp = pool("sq2s", 1, space="PSUM")

    x2, x2b = [], []
    S2_ps = s2sp.tile([128, T], dtf, tag="S2")
    for e in range(ET):
        pr_ps = prsp.tile([128, T], dtf, tag="prs")
        for i in range(ET // 2):
            mm(pr_ps[:], wo8_sb[i][:, :, ts(e, 128)], oall8[i][:, :, :],
               start=(i == 0), stop=(i == ET // 2 - 1), perf_mode=DR)
        tx2 = x2p.tile([128, T], dtf, tag="x2")
        nc.vector.scalar_tensor_tensor(tx2[:], pr_ps[:], sc_s[:, 2 + e:3 + e],
                                       xo_sb[e][:], ALU.mult, ALU.add)
        x2.append(tx2)
        tb = x2bp.tile([128, T], dtb, tag="x2b")
        nc.scalar.activation(tb[:], tx2[:], AF.Copy)
        x2b.append(tb)
        mm(S2_ps[:], ones128[:], tb[:], start=(e == 0), stop=(e == ET - 1))

    h2 = [h2p.tile([128, T], dtb, tag="h2", name=f"h2_{e}")
          for e in range(ET)]
    sq2 = squares(sqdp, [t[:, :] for t in x2b], T)
    SQ2r = pe_sum(sq2sp, "SQ2", [t[:] for t in sq2], T)
    ln_normalize(abdp, [t[:, :] for t in x2b], S2_ps, SQ2r,
                 [t[:, :] for t in h2], T)
    close(sq2s_cm, s2s_cm, prs_cm, abd_cm, sqd_cm, x2b_cm)
    close(oall_cm, wo8_cm)
    if stage_limit == "D":
        close(h2_cm, x2_cm, xo_cm, w1l_cm, w1h_cm, w1b_cm, w1a_cm, cpool_cm)
        return

    # ============ stage E: FFN ============
    # FFN1 bf16; FFN2 entirely as fp8 weight-residual DoubleRow: hi and lo
    # chains share one per-column scale (psum descales once per e-tile),
    # gelu writes g directly in unscaled e4m3.
    FP = FT // 2                         # 16 f-pair tiles
    FQ = FT - FT // 4                    # first bf16 f-tile count (24)
    h28_cm, h28p = pool("h28", ET // 2)
    g8_cm, g8p = pool("g8", FP)
    w2h_cm, w2hp = pool("w2h", FP)
    w2l_cm, w2lp = pool("w2l", FP)
    w2h_sb, w2l_sb = [], []
    for i in range(FP):
        th = w2hp.tile([128, 2, D], dt8, tag="w2h")
        nc.sync.dma_start(out=th[:], in_=w2h8_d[:, ts(i, 2 * D)])
        w2h_sb.append(th)
    for i in range(FP):
        tl = w2lp.tile([128, 2, D], dt8, tag="w2l")
        nc.sync.dma_start(out=tl[:], in_=w2l8_d[:, ts(i, 2 * D)])
        w2l_sb.append(tl)
    aps_cm, apsp = pool("aps", 3, space="PSUM")
    g8_sb = [g8p.tile([128, 2, T], dt8, tag="g8", name=f"g8_{i}")
             for i in range(FP)]
    # fp8 copy of h2 for the residual FFN1 tail tiles (ACT, ahead of the
    # gelu queue so it lands while the bf16 chains run)
    h28 = [h28p.tile([128, 2, T], dt8, tag="h28", name=f"h28_{i}")
           for i in range(ET // 2)]
    for i in range(ET // 2):
        for jb in range(2):
            nc.scalar.activation(h28[i][:, jb, :], h2[2 * i + jb][:],
                                 AF.Copy, scale=H8S)
    for f in range(FT):
        a_ps = apsp.tile([128, T], dtf, tag="aps")
        if f < FQ:
            for e in range(ET):
                mm(a_ps[:], w1_sb[e][:, ts(f, 128)], h2[e][:],
                   start=(e == 0), stop=(e == ET - 1))
            nc.scalar.activation(g8_sb[f // 2][:, f % 2, :], a_ps[:],
                                 AF.Gelu, bias=b1e_s[:, f:f + 1])
        else:
            fr = f - FQ
            for i in range(ET // 2):
                mm(a_ps[:], w1h_sb[i][:, :, ts(fr, 128)], h28[i][:, :, :],
                   start=(i == 0), stop=False, perf_mode=DR)
            for i in range(ET // 2):
                mm(a_ps[:], w1l_sb[i][:, :, ts(fr, 128)], h28[i][:, :, :],
                   start=False, stop=(i == ET // 2 - 1), perf_mode=DR)
            nc.scalar.activation(g8_sb[f // 2][:, f % 2, :], a_ps[:],
                                 AF.Gelu, bias=b1e_s[:, f:f + 1],
                                 scale=sc_s[:, 2 + 2 * ET + fr:3 + 2 * ET + fr])
    close(aps_cm)

    ob_cm, obp = pool("ob", 4)
    yps_cm, ypsp = pool("yps", 3, space="PSUM")
    for e in range(ET):
        y_ps = ypsp.tile([128, T], dtf, tag="yps")
        for i in range(FP):
            mm(y_ps[:], w2h_sb[i][:, :, ts(e, 128)], g8_sb[i][:, :, :],
               start=(i == 0), stop=False, perf_mode=DR)
        for i in range(FP):
            mm(y_ps[:], w2l_sb[i][:, :, ts(e, 128)], g8_sb[i][:, :, :],
               start=False, stop=(i == FP - 1), perf_mode=DR)
        ob = obp.tile([128, T], dtf, tag="ob")
        nc.vector.scalar_tensor_tensor(ob[:], y_ps[:],
                                       sc_s[:, 2 + ET + e:3 + ET + e],
                                       x2[e][:], ALU.mult, ALU.add)
        if not zero_bias:
            nc.vector.tensor_scalar_add(ob[:], ob[:], b2_s[:, e:e + 1])
        nc.sync.dma_start(out=outT_d[ts(e, 128), :], in_=ob[:])
    close(yps_cm, ob_cm, w2l_cm, w2h_cm, g8_cm, h28_cm)

    close(h2_cm, x2_cm, xo_cm, w1l_cm, w1h_cm, w1b_cm, w1a_cm, cpool_cm)


_NC_CACHE = {}


def _zero_bias_flag(ln1_b):
    return bool(np.all(np.asarray(ln1_b) == 0.0))


def _get_nc(zero_bias=False):
    key = ("nc", zero_bias)
    if key not in _NC_CACHE:
        _NC_CACHE[key] = build(zero_bias=zero_bias)
    return _NC_CACHE[key]


def _vec_tiles(v, ntiles):
    return np.ascontiguousarray(
        np.asarray(v, np.float32).reshape(ntiles, 128).T)


def _pair_blocks(w8):
    """[K, cols] quantized array -> [128, (K//256)*2*cols] pair-block layout."""
    blocks = []
    for i in range(w8.shape[0] // 256):
        for j in range(2):
            blocks.append(w8[(2 * i + j) * 128:(2 * i + j + 1) * 128, :])
    return np.ascontiguousarray(np.concatenate(blocks, axis=1))


def _fp8_pairs(w, colscale):
    """[D, D] fp32 -> [128, (D//256)*2*D] e4m3 pair-block layout."""
    return _pair_blocks((w * colscale).astype(ml_dtypes.float8_e4m3))


def prepare_in_maps(x, wq, wk, wv, wo, w1, b1, w2, b2,
                    ln1_g, ln1_b, ln2_g, ln2_b):
    bf = ml_dtypes.bfloat16
    f32 = np.float32
    x = np.asarray(x, f32)
    wq = np.asarray(wq, f32); wk = np.asarray(wk, f32)
    wv = np.asarray(wv, f32); w1 = np.asarray(w1, f32)
    wo = np.asarray(wo, f32)
    g1 = np.asarray(ln1_g, f32)[:, None]
    b1v = np.asarray(ln1_b, f32)
    g2 = np.asarray(ln2_g, f32)[:, None]
    b2v = np.asarray(ln2_b, f32)
    bq = (b1v @ wq).astype(f32)          # [D] per-output-col constants
    bk = (b1v @ wk).astype(f32)
    bv = (b1v @ wv).astype(f32)
    b1eff = (np.asarray(b1, f32) + b2v @ w1).astype(f32)

    wv_g = wv * g1
    s_wv = 120.0 / max(1e-30, np.abs(wv_g).max())
    # bound on |attn out| <= max |v| row; scale so fp8 oall stays in range
    vbound = 6.0 * np.linalg.norm(wv_g, axis=0).max() + np.abs(bv).max()
    s_oall = 120.0 / max(1e-30, vbound)
    s_wocol = 120.0 / np.maximum(np.abs(wo).max(axis=0), 1e-30)
    # FFN2 split: rows 0..FF/2 as fp8 hi+lo residual, rest bf16 pre-scaled
    # by the shared per-column scale s2col (g stays in unscaled e4m3)
    w2f = np.asarray(w2, f32)
    s2col = 120.0 / np.maximum(np.abs(w2f).max(axis=0), 1e-30)
    f8t = ml_dtypes.float8_e4m3
    w2s = w2f * s2col[None, :]
    w2hi = np.asarray(w2s, f8t)
    w2lo = np.asarray(w2s - w2hi.astype(f32), f8t)
    # FFN1 residual tail (f-cols 3072..4095): hi+lo fp8 with per-col scale
    w1_g = np.asarray(w1, f32) * g2
    s1col = 120.0 / np.maximum(np.abs(w1_g[:, 3 * FF // 4:]).max(axis=0),
                               1e-30)
    w1s = w1_g[:, 3 * FF // 4:] * s1col[None, :]
    w1hi = np.asarray(w1s, f8t)
    w1lo = np.asarray(w1s - w1hi.astype(f32), f8t)
    sc = np.zeros((128, 2 + 2 * ET + FT // 4), f32)
    for fr in range(FT // 4):
        sc[:, 2 + 2 * ET + fr] = 1.0 / (H8S * s1col[fr * 128:(fr + 1) * 128])
    sc[:, 0] = 1.0 / (H8S * s_wv)
    sc[:, 1] = s_oall
    for e in range(ET):
        sc[:, 2 + e] = 1.0 / (s_oall * s_wocol[e * 128:(e + 1) * 128])
        sc[:, 2 + ET + e] = 1.0 / s2col[e * 128:(e + 1) * 128]

    shared = {
        "wq": np.ascontiguousarray((wq * g1).astype(bf)),
        "wk": np.ascontiguousarray((wk * g1).astype(bf)),
        "wv8": _fp8_pairs(wv_g, s_wv),
        "wo8": _fp8_pairs(wo, s_wocol[None, :]),
        "w1": np.ascontiguousarray(w1_g[:, :3 * FF // 4].astype(bf)),
        "w1h8": _pair_blocks(w1hi),
        "w1l8": _pair_blocks(w1lo),
        "w2h8": _pair_blocks(w2hi),
        "w2l8": _pair_blocks(w2lo),
        "bq": np.ascontiguousarray(bq.reshape(HP, 128).T),
        "bk": np.ascontiguousarray(bk.reshape(HP, 128).T),
        "bvw": np.ascontiguousarray(np.tile(bv.astype(bf), (128, 1))),
        "b1e": _vec_tiles(b1eff, FT),
        "b2": _vec_tiles(b2, ET),
        "ident": np.ascontiguousarray(np.eye(128, dtype=f32)),
        "identb": np.ascontiguousarray(np.eye(128, dtype=bf)),
        "sc": np.ascontiguousarray(sc),
    }
    in_maps = []
    for c in range(NCORES):
        b, s = divmod(c, CPB)
        rot = np.concatenate([x[b, s * T:], x[b, :s * T]], axis=0)  # own first
        m = dict(shared)
        m["xbT"] = np.ascontiguousarray(rot.T.astype(bf))
        m["xoT"] = np.ascontiguousarray(x[b, s * T:(s + 1) * T].T.astype(bf))
        in_maps.append(m)
    return in_maps


def assemble_output(results):
    out = np.empty((B, N, D), np.float32)
    for c in range(NCORES):
        b, s = divmod(c, CPB)
        out[b, s * T:(s + 1) * T, :] = results[c]["outT"].T
    return out


def kernel(x, wq, wk, wv, wo, w1, b1, w2, b2, ln1_g, ln1_b, ln2_g, ln2_b):
    from concourse.bass_utils import run_bass_kernel_spmd

    nc = _get_nc(_zero_bias_flag(ln1_b))
    in_maps = prepare_in_maps(x, wq, wk, wv, wo, w1, b1, w2, b2,
                              ln1_g, ln1_b, ln2_g, ln2_b)
    res = run_bass_kernel_spmd(nc, in_maps, core_ids=list(range(NCORES)))
    return assemble_output(res.results)

